# revision 1
# baseline (speedup 1.0000x reference)
"""MoE layer (moe_routing) Trainium2 Bass kernel — 8-core expert parallelism.

Strategy (hardcoded for T=8192, D=1024, F=2048, E=8, top_k=2, 8 cores):
  - Core e owns expert e's w1/w3/w2, plus a 256-wide F-slice of the shared expert.
  - hidden_states is replicated to every core (row-major `x` for token gathers and
    host-transposed `xT` for matmul rhs layout).
  - Router is token-sharded: core r routes tokens [1024r, 1024(r+1)) in float32r
    (near-fp32 PE precision), then an AllGather shares the per-expert combine
    weights + shared-expert gate with everyone.
  - top-2 renormalized softmax weights are computed as sigmoid(l1-l2) and
    1-sigmoid(l1-l2) (exact reformulation), using the DVE max/max_index top-8
    sort instructions.
  - Each core compacts its expert's token ids with a cumsum-by-triangular-matmul
    and indirect-DMA scatters, gathers those token rows, runs the FFN in bf16,
    and indirect-DMA scatter-adds weighted rows into a [T, D] bf16 partial that
    is also scatter-added (static iota offsets) with the gated shared-expert
    F-slice output.
  - A ReduceScatter(add) over the 8 cores combines partials; each core emits the
    final f32 output for its 1024-token slice; the host concatenates.
"""
import sys

sys.path.insert(0, "/opt/trn_rl_repo")

import numpy as np

import concourse.bacc as bacc
import concourse.mybir as mybir
import concourse.tile as tile
from concourse.bass import IndirectOffsetOnAxis
from concourse.bass_utils import run_bass_kernel_spmd
from concourse.masks import make_identity

dt = mybir.dt
AF = mybir.ActivationFunctionType
OP = mybir.AluOpType

P = 128
T, D, F, E = 8192, 1024, 2048, 8
FS = F // 8          # shared-expert F slice per core
C = 2560             # expert token capacity per core (max measured load 2182)
TB = 512             # token block
NBT = T // TB        # 16 shared-expert blocks
NBC = T // P         # 64 token chunks
NBF = C // TB        # 5 expert FFN blocks
TSL = T // 8         # 1024 router tokens per core
BIG = 1 << 20
RG = [list(range(8))]

_CACHE = {}


def _build():
    if "nc" in _CACHE:
        return _CACHE["nc"]
    nc = bacc.Bacc("TRN2", target_bir_lowering=False, debug=False, num_devices=8)

    x_ext = nc.dram_tensor("x", [T, D], dt.float32, kind="ExternalInput")
    xT_ext = nc.dram_tensor("xT", [D, T], dt.float32, kind="ExternalInput")
    xTr_ext = nc.dram_tensor("xTr", [D, TSL], dt.float32, kind="ExternalInput")
    gw9_ext = nc.dram_tensor("gw9", [D, 9], dt.float32, kind="ExternalInput")
    w1_ext = nc.dram_tensor("w1e", [D, F], dt.float32, kind="ExternalInput")
    w3_ext = nc.dram_tensor("w3e", [D, F], dt.float32, kind="ExternalInput")
    w2_ext = nc.dram_tensor("w2e", [F, D], dt.float32, kind="ExternalInput")
    sw1_ext = nc.dram_tensor("sw1e", [D, FS], dt.float32, kind="ExternalInput")
    sw3_ext = nc.dram_tensor("sw3e", [D, FS], dt.float32, kind="ExternalInput")
    sw2_ext = nc.dram_tensor("sw2e", [FS, D], dt.float32, kind="ExternalInput")
    eoh_ext = nc.dram_tensor("eoh", [P, E], dt.float32, kind="ExternalInput")
    out_ext = nc.dram_tensor("out", [TSL, D], dt.float32, kind="ExternalOutput")

    with tile.TileContext(nc) as tc:
        with tc.tile_pool(name="cn", bufs=1) as cn, \
             tc.tile_pool(name="wk", bufs=2) as wk, \
             tc.tile_pool(name="ps", bufs=1, space="PSUM") as ps, \
             tc.tile_pool(name="dr", bufs=1, space="DRAM") as dr:

            # ---------------- DRAM scratch ----------------
            cwslice = dr.tile([TSL, 9], dt.float32)
            cwfull = dr.tile([T, 9], dt.float32, addr_space="Shared")
            iw_dram = dr.tile([C, 2], dt.int32)
            partial = dr.tile([T, D], dt.bfloat16)
            rsout = dr.tile([TSL, D], dt.bfloat16)

            # ---------------- constants ----------------
            ident_bf = cn.tile([P, P], dt.bfloat16)
            make_identity(nc, ident_bf[:])
            ident_f = cn.tile([P, P], dt.float32)
            make_identity(nc, ident_f[:])
            ones_bf = cn.tile([P, P], dt.bfloat16)
            nc.vector.memset(ones_bf[:], 1.0)
            # tri[k, m] = 1 if k < m (strictly-lower in (k,m)): m - k - 1 >= 0
            tri_bf = cn.tile([P, P], dt.bfloat16)
            nc.gpsimd.affine_select(
                out=tri_bf[:], in_=ones_bf[:], pattern=[[1, P]], base=-1,
                channel_multiplier=-1, compare_op=OP.is_ge, fill=0.0)
            ones_row_f = cn.tile([1, P], dt.float32)
            nc.vector.memset(ones_row_f[:], 1.0)
            iota8_i = cn.tile([P, E], dt.int32)
            nc.gpsimd.iota(iota8_i[:], pattern=[[1, E]], base=0, channel_multiplier=0)
            iota8_f = cn.tile([P, E], dt.float32)
            nc.vector.tensor_copy(out=iota8_f[:], in_=iota8_i[:])
            iota64 = cn.tile([P, NBC], dt.int32)
            nc.gpsimd.iota(iota64[:], pattern=[[P, NBC]], base=0, channel_multiplier=1)
            eoh_sb = cn.tile([P, E], dt.float32)
            nc.sync.dma_start(out=eoh_sb[:], in_=eoh_ext[:, :])

            # ---------------- zero-init partial + iw ----------------
            zb = cn.tile([P, D], dt.bfloat16)
            nc.vector.memset(zb[:], 0.0)
            pr = partial[:, :].rearrange("(a p) f -> p a f", p=P)  # [128, 64, 1024]
            for g in range(NBC):
                nc.sync.dma_start(out=pr[:, g, :], in_=zb[:])
            zi = cn.tile([P, C // P, 2], dt.int32)
            nc.vector.memset(zi[:], 0)
            nc.sync.dma_start(
                out=iw_dram[:, :].rearrange("(a p) f -> p a f", p=P), in_=zi[:])

            # ---------------- resident weights (bf16) ----------------
            gw9s = cn.tile([P, E, 9], dt.float32r)
            for k in range(E):
                nc.sync.dma_start(
                    out=gw9s[:, k, :],
                    in_=gw9_ext[k * P:(k + 1) * P, :].bitcast(dt.float32r))

            w1s = cn.tile([P, 8, F], dt.bfloat16)
            w3s = cn.tile([P, 8, F], dt.bfloat16)
            w2s = cn.tile([P, 16, D], dt.bfloat16)
            sw1s = cn.tile([P, 8, FS], dt.bfloat16)
            sw3s = cn.tile([P, 8, FS], dt.bfloat16)
            sw2s = cn.tile([P, 2, D], dt.bfloat16)

            def load_w(dst, src, k, width, eng):
                for j in range(0, width, D):
                    w = min(D, width - j)
                    stg = wk.tile([P, D], dt.float32, tag="wstg", bufs=2, name="wstg")
                    nc.sync.dma_start(out=stg[:, :w],
                                      in_=src[k * P:(k + 1) * P, j:j + w])
                    if eng == "v":
                        nc.vector.tensor_copy(out=dst[:, k, j:j + w], in_=stg[:, :w])
                    else:
                        nc.scalar.activation(out=dst[:, k, j:j + w], in_=stg[:, :w],
                                             func=AF.Copy)

            for k in range(8):
                load_w(w1s, w1_ext, k, F, "v")
                load_w(w3s, w3_ext, k, F, "s")
            for k in range(16):
                load_w(w2s, w2_ext, k, D, "v")
            for k in range(8):
                load_w(sw1s, sw1_ext, k, FS, "v")
                load_w(sw3s, sw3_ext, k, FS, "s")
            for k in range(2):
                load_w(sw2s, sw2_ext, k, D, "v")

            # ---------------- phase 1: router on local token slice ----------------
            payload = cn.tile([P, TSL // P, 9], dt.float32)
            for tb in range(TSL // TB):
                psl = ps.tile([9, TB], dt.float32, tag="small", bufs=2, name="psl")
                for k in range(8):
                    xtr = wk.tile([P, TB], dt.float32r, bufs=2, name="xtr")
                    nc.sync.dma_start(
                        out=xtr[:],
                        in_=xTr_ext[k * P:(k + 1) * P, tb * TB:(tb + 1) * TB]
                        .bitcast(dt.float32r))
                    nc.tensor.matmul(out=psl[:], lhsT=gw9s[:, k, :], rhs=xtr[:],
                                     start=(k == 0), stop=(k == 7))
                lsb = wk.tile([9, TB], dt.float32, bufs=2, name="lsb")
                nc.vector.tensor_copy(out=lsb[:], in_=psl[:])
                for a in range(4):
                    c_loc = tb * 4 + a
                    pstt = ps.tile([P, 9], dt.float32, tag="small", bufs=2, name="pstt")
                    nc.tensor.transpose(out=pstt[:], in_=lsb[:, a * P:(a + 1) * P],
                                        identity=ident_f[:9, :9])
                    lgc = wk.tile([P, 9], dt.float32, bufs=2, name="lgc")
                    nc.vector.tensor_copy(out=lgc[:], in_=pstt[:])
                    mx = wk.tile([P, 8], dt.float32, bufs=2, name="mx")
                    nc.vector.max(out=mx[:], in_=lgc[:, 0:8])
                    mi = wk.tile([P, 8], dt.uint32, bufs=2, name="mi")
                    nc.vector.max_index(out=mi[:], in_max=mx[:], in_values=lgc[:, 0:8])
                    mif = wk.tile([P, 2], dt.float32, bufs=2, name="mif")
                    nc.vector.tensor_copy(out=mif[:], in_=mi[:, 0:2].bitcast(dt.int32))
                    d12 = wk.tile([P, 1], dt.float32, bufs=2, name="d12")
                    nc.vector.tensor_sub(d12[:], mx[:, 0:1], mx[:, 1:2])
                    wA = wk.tile([P, 1], dt.float32, bufs=2, name="wA")
                    nc.scalar.activation(out=wA[:], in_=d12[:], func=AF.Sigmoid)
                    wB = wk.tile([P, 1], dt.float32, bufs=2, name="wB")
                    nc.scalar.activation(out=wB[:], in_=wA[:], func=AF.Copy,
                                         scale=-1.0, bias=1.0)
                    eq1 = wk.tile([P, 8], dt.float32, bufs=2, name="eq1")
                    nc.vector.tensor_tensor(
                        out=eq1[:], in0=mif[:, 0:1].to_broadcast([P, 8]),
                        in1=iota8_f[:], op=OP.is_equal)
                    eq2 = wk.tile([P, 8], dt.float32, bufs=2, name="eq2")
                    nc.vector.tensor_tensor(
                        out=eq2[:], in0=mif[:, 1:2].to_broadcast([P, 8]),
                        in1=iota8_f[:], op=OP.is_equal)
                    nc.vector.tensor_tensor(out=eq1[:], in0=eq1[:],
                                            in1=wA[:].to_broadcast([P, 8]), op=OP.mult)
                    nc.vector.tensor_tensor(out=eq2[:], in0=eq2[:],
                                            in1=wB[:].to_broadcast([P, 8]), op=OP.mult)
                    nc.vector.tensor_add(payload[:, c_loc, 0:8], eq1[:], eq2[:])
                    nc.scalar.activation(out=payload[:, c_loc, 8:9], in_=lgc[:, 8:9],
                                         func=AF.Sigmoid)
            nc.sync.dma_start(
                out=cwslice[:, :].rearrange("(c p) f -> p c f", p=P), in_=payload[:])
            nc.gpsimd.collective_compute(
                "AllGather", OP.bypass, replica_groups=RG,
                ins=[cwslice[:, :].opt()], outs=[cwfull[:, :].opt()])

            # ---------------- phase 2: masks + compaction ----------------
            cwe_all = cn.tile([P, NBC], dt.float32)
            gate_all = cn.tile([P, NBC], dt.float32)
            for g in range(8):  # 8 groups of 8 chunks
                cwg = wk.tile([P, 8, 9], dt.float32, bufs=2, name="cwg")
                nc.sync.dma_start(
                    out=cwg[:],
                    in_=cwfull[g * 1024:(g + 1) * 1024, :]
                    .rearrange("(c p) f -> p c f", p=P))
                for j in range(8):
                    c = g * 8 + j
                    pr8 = wk.tile([P, 8], dt.float32, bufs=2, name="pr8")
                    nc.vector.tensor_tensor(out=pr8[:], in0=cwg[:, j, 0:8],
                                            in1=eoh_sb[:], op=OP.mult)
                    nc.vector.reduce_sum(cwe_all[:, c:c + 1], pr8[:],
                                         axis=mybir.AxisListType.X)
                    nc.vector.tensor_copy(out=gate_all[:, c:c + 1], in_=cwg[:, j, 8:9])
            mask_f = cn.tile([P, NBC], dt.float32)
            nc.vector.tensor_scalar(out=mask_f[:], in0=cwe_all[:], scalar1=0.0,
                                    scalar2=None, op0=OP.is_gt)
            mask_bf = cn.tile([P, NBC], dt.bfloat16)
            nc.vector.tensor_copy(out=mask_bf[:], in_=mask_f[:])

            # column sums -> exclusive prefix over the 64 columns
            pcst = ps.tile([P, 1], dt.float32, tag="small", bufs=2, name="pcst")
            nc.tensor.matmul(out=pcst[0:NBC, :], lhsT=mask_bf[:], rhs=ones_bf[:, 0:1],
                             start=True, stop=True)
            cst = wk.tile([NBC, 1], dt.bfloat16, bufs=2, name="cst")
            nc.vector.tensor_copy(out=cst[:], in_=pcst[0:NBC, :])
            ppre = ps.tile([P, 1], dt.float32, tag="small", bufs=2, name="ppre")
            nc.tensor.matmul(out=ppre[0:NBC, :], lhsT=tri_bf[0:NBC, 0:NBC], rhs=cst[:],
                             start=True, stop=True)
            pre_sb = wk.tile([NBC, 1], dt.float32, bufs=2, name="pre_sb")
            nc.vector.tensor_copy(out=pre_sb[:], in_=ppre[0:NBC, :])
            pprer = ps.tile([1, NBC], dt.float32, tag="small", bufs=2, name="pprer")
            nc.tensor.transpose(out=pprer[:], in_=pre_sb[:],
                                identity=ident_f[0:NBC, 0:NBC])
            pre_row = wk.tile([1, NBC], dt.float32, bufs=2, name="pre_row")
            nc.vector.tensor_copy(out=pre_row[:], in_=pprer[:])

            # pos = within-column exclusive cumsum + column prefix (PSUM accumulate)
            ppos = ps.tile([P, NBC], dt.float32, tag="small", bufs=2, name="ppos")
            nc.tensor.matmul(out=ppos[:], lhsT=tri_bf[:], rhs=mask_bf[:],
                             start=True, stop=False)
            nc.tensor.matmul(out=ppos[:], lhsT=ones_row_f[:], rhs=pre_row[:],
                             start=False, stop=True)
            posm = wk.tile([P, NBC], dt.float32, bufs=2, name="posm")
            nc.vector.tensor_tensor(out=posm[:], in0=ppos[:], in1=mask_f[:], op=OP.mult)
            dump = wk.tile([P, NBC], dt.float32, bufs=2, name="dump")
            nc.vector.tensor_scalar(out=dump[:], in0=mask_f[:], scalar1=float(-BIG),
                                    scalar2=float(BIG), op0=OP.mult, op1=OP.add)
            nc.vector.tensor_add(posm[:], posm[:], dump[:])
            o_i = cn.tile([P, NBC], dt.int32)
            nc.vector.tensor_copy(out=o_i[:], in_=posm[:])

            iw_pack = cn.tile([P, NBC, 2], dt.int32)
            nc.vector.tensor_copy(out=iw_pack[:, :, 0], in_=iota64[:])
            nc.vector.tensor_copy(out=iw_pack[:, :, 1], in_=cwe_all[:].bitcast(dt.int32))
            for c in range(NBC):
                nc.gpsimd.indirect_dma_start(
                    out=iw_dram[:, :],
                    out_offset=IndirectOffsetOnAxis(ap=o_i[:, c:c + 1], axis=0),
                    in_=iw_pack[:, c, :], in_offset=None,
                    bounds_check=C - 1, oob_is_err=False)

            # ---------------- phase 3: expert FFN on compacted tokens ----------------
            for b in range(NBF):
                iw_sb = wk.tile([P, 4, 2], dt.int32, bufs=2, name="iw_sb")
                nc.sync.dma_start(
                    out=iw_sb[:],
                    in_=iw_dram[b * TB:(b + 1) * TB, :]
                    .rearrange("(a p) f -> p a f", p=P))
                xcT = wk.tile([P, 8, TB], dt.bfloat16, bufs=1, name="xcT")
                for a in range(4):
                    xg = wk.tile([P, D], dt.float32, bufs=2, name="xg")
                    nc.gpsimd.indirect_dma_start(
                        out=xg[:], out_offset=None, in_=x_ext[:, :],
                        in_offset=IndirectOffsetOnAxis(ap=iw_sb[:, a, 0:1], axis=0))
                    xg_bf = wk.tile([P, D], dt.bfloat16, bufs=2, name="xg_bf")
                    nc.vector.tensor_copy(out=xg_bf[:], in_=xg[:])
                    for k in range(8):
                        psxt = ps.tile([P, P], dt.bfloat16, tag="small", bufs=2,
                                       name="psxt")
                        nc.tensor.transpose(out=psxt[:],
                                            in_=xg_bf[:, k * P:(k + 1) * P],
                                            identity=ident_bf[:])
                        nc.vector.tensor_copy(out=xcT[:, k, a * P:(a + 1) * P],
                                              in_=psxt[:])
                hs = wk.tile([P, 16, TB], dt.bfloat16, bufs=1, name="hs")
                for fk in range(16):
                    ph1 = ps.tile([P, TB], dt.float32, tag="mm512", bufs=2, name="ph1")
                    for k in range(8):
                        nc.tensor.matmul(out=ph1[:], lhsT=w1s[:, k, fk * P:(fk + 1) * P],
                                         rhs=xcT[:, k, :], start=(k == 0), stop=(k == 7))
                    ph3 = ps.tile([P, TB], dt.float32, tag="mm512", bufs=2, name="ph3")
                    for k in range(8):
                        nc.tensor.matmul(out=ph3[:], lhsT=w3s[:, k, fk * P:(fk + 1) * P],
                                         rhs=xcT[:, k, :], start=(k == 0), stop=(k == 7))
                    hg = wk.tile([P, TB], dt.bfloat16, bufs=2, name="hg")
                    nc.scalar.activation(out=hg[:], in_=ph1[:], func=AF.Silu)
                    h3b = wk.tile([P, TB], dt.bfloat16, bufs=2, name="h3b")
                    nc.vector.tensor_copy(out=h3b[:], in_=ph3[:])
                    nc.vector.tensor_mul(hs[:, fk, :], hg[:], h3b[:])
                psa = [ps.tile([P, D], dt.bfloat16, tag="otr", bufs=4, name="psa")
                       for _ in range(4)]
                for k2 in range(8):
                    po = ps.tile([P, TB], dt.float32, tag="mm512", bufs=2, name="po")
                    for fk in range(16):
                        nc.tensor.matmul(out=po[:], lhsT=w2s[:, fk, k2 * P:(k2 + 1) * P],
                                         rhs=hs[:, fk, :], start=(fk == 0), stop=(fk == 15))
                    ob = wk.tile([P, TB], dt.bfloat16, bufs=2, name="ob")
                    nc.scalar.activation(out=ob[:], in_=po[:], func=AF.Copy)
                    for a in range(4):
                        nc.tensor.transpose(out=psa[a][:, k2 * P:(k2 + 1) * P],
                                            in_=ob[:, a * P:(a + 1) * P],
                                            identity=ident_bf[:])
                for a in range(4):
                    otw = wk.tile([P, D], dt.bfloat16, bufs=1, name="otw")
                    nc.vector.tensor_scalar_mul(otw[:], psa[a][:],
                                                iw_sb[:, a, 1:2].bitcast(dt.float32))
                    nc.gpsimd.indirect_dma_start(
                        out=partial[:, :],
                        out_offset=IndirectOffsetOnAxis(ap=iw_sb[:, a, 0:1], axis=0),
                        in_=otw[:], in_offset=None,
                        bounds_check=T - 1, oob_is_err=False,
                        compute_op=OP.add)

            # ---------------- phase 4: shared expert (F-slice), gated ----------------
            for tb in range(NBT):
                xts = wk.tile([P, 8, TB], dt.bfloat16, bufs=1, name="xts")
                for k in range(8):
                    xstg = wk.tile([P, TB], dt.float32, bufs=2, name="xstg")
                    nc.sync.dma_start(
                        out=xstg[:], in_=xT_ext[k * P:(k + 1) * P, tb * TB:(tb + 1) * TB])
                    nc.vector.tensor_copy(out=xts[:, k, :], in_=xstg[:])
                ss = wk.tile([P, 2, TB], dt.bfloat16, bufs=1, name="ss")
                for fs in range(2):
                    ps1 = ps.tile([P, TB], dt.float32, tag="mm512", bufs=2, name="ps1")
                    for k in range(8):
                        nc.tensor.matmul(out=ps1[:], lhsT=sw1s[:, k, fs * P:(fs + 1) * P],
                                         rhs=xts[:, k, :], start=(k == 0), stop=(k == 7))
                    ps3 = ps.tile([P, TB], dt.float32, tag="mm512", bufs=2, name="ps3")
                    for k in range(8):
                        nc.tensor.matmul(out=ps3[:], lhsT=sw3s[:, k, fs * P:(fs + 1) * P],
                                         rhs=xts[:, k, :], start=(k == 0), stop=(k == 7))
                    sgs = wk.tile([P, TB], dt.bfloat16, bufs=2, name="sgs")
                    nc.scalar.activation(out=sgs[:], in_=ps1[:], func=AF.Silu)
                    s3b = wk.tile([P, TB], dt.bfloat16, bufs=2, name="s3b")
                    nc.vector.tensor_copy(out=s3b[:], in_=ps3[:])
                    nc.vector.tensor_mul(ss[:, fs, :], sgs[:], s3b[:])
                pst = [ps.tile([P, D], dt.bfloat16, tag="otr", bufs=4, name="pst")
                       for _ in range(4)]
                for k2 in range(8):
                    pso2 = ps.tile([P, TB], dt.float32, tag="mm512", bufs=2, name="pso2")
                    for fs in range(2):
                        nc.tensor.matmul(out=pso2[:], lhsT=sw2s[:, fs, k2 * P:(k2 + 1) * P],
                                         rhs=ss[:, fs, :], start=(fs == 0), stop=(fs == 1))
                    sob = wk.tile([P, TB], dt.bfloat16, bufs=2, name="sob")
                    nc.scalar.activation(out=sob[:], in_=pso2[:], func=AF.Copy)
                    for a in range(4):
                        nc.tensor.transpose(out=pst[a][:, k2 * P:(k2 + 1) * P],
                                            in_=sob[:, a * P:(a + 1) * P],
                                            identity=ident_bf[:])
                for a in range(4):
                    c = tb * 4 + a
                    stg = wk.tile([P, D], dt.bfloat16, bufs=2, name="stg")
                    nc.vector.tensor_scalar_mul(stg[:], pst[a][:], gate_all[:, c:c + 1])
                    nc.gpsimd.indirect_dma_start(
                        out=partial[:, :],
                        out_offset=IndirectOffsetOnAxis(ap=iota64[:, c:c + 1], axis=0),
                        in_=stg[:], in_offset=None,
                        bounds_check=T - 1, oob_is_err=False,
                        compute_op=OP.add)

            # ---------------- phase 5: ReduceScatter + output ----------------
            nc.gpsimd.collective_compute(
                "ReduceScatter", OP.add, replica_groups=RG,
                ins=[partial[:, :].opt()], outs=[rsout[:, :].opt()])
            for k in range(TSL // P):
                rsb = wk.tile([P, D], dt.bfloat16, bufs=1, name="rsb")
                nc.sync.dma_start(out=rsb[:], in_=rsout[k * P:(k + 1) * P, :])
                rsf = wk.tile([P, D], dt.float32, bufs=1, name="rsf")
                nc.vector.tensor_copy(out=rsf[:], in_=rsb[:])
                nc.sync.dma_start(out=out_ext[k * P:(k + 1) * P, :], in_=rsf[:])

    nc.compile()
    _CACHE["nc"] = nc
    return nc


def _shard(inputs):
    x = np.ascontiguousarray(np.asarray(inputs["hidden_states"], dtype=np.float32))
    xT = np.ascontiguousarray(x.T)
    gw9 = np.ascontiguousarray(
        np.concatenate([np.asarray(inputs["gate_w"], np.float32),
                        np.asarray(inputs["sgate_w"], np.float32)], axis=1))
    w1 = np.asarray(inputs["w1"], np.float32)
    w3 = np.asarray(inputs["w3"], np.float32)
    w2 = np.asarray(inputs["w2"], np.float32)
    sw1 = np.asarray(inputs["sw1"], np.float32)
    sw3 = np.asarray(inputs["sw3"], np.float32)
    sw2 = np.asarray(inputs["sw2"], np.float32)
    in_maps = []
    for r in range(8):
        eoh = np.zeros((P, E), np.float32)
        eoh[:, r] = 1.0
        in_maps.append(dict(
            x=x,
            xT=xT,
            xTr=np.ascontiguousarray(xT[:, r * TSL:(r + 1) * TSL]),
            gw9=gw9,
            w1e=np.ascontiguousarray(w1[r]),
            w3e=np.ascontiguousarray(w3[r]),
            w2e=np.ascontiguousarray(w2[r]),
            sw1e=np.ascontiguousarray(sw1[:, r * FS:(r + 1) * FS]),
            sw3e=np.ascontiguousarray(sw3[:, r * FS:(r + 1) * FS]),
            sw2e=np.ascontiguousarray(sw2[r * FS:(r + 1) * FS, :]),
            eoh=eoh,
        ))
    return in_maps


def run(inputs, trace=False):
    nc = _build()
    in_maps = _shard(inputs)
    res = run_bass_kernel_spmd(nc, in_maps, list(range(8)), trace=trace)
    out = np.concatenate([res.results[r]["out"] for r in range(8)], axis=0)
    return out.astype(np.float32), res


def kernel(**inputs):
    out, _ = run(inputs, trace=False)
    return out



# revision 17
# speedup vs baseline: 1.4756x; 1.4756x over previous
"""MoE layer (moe_routing) Trainium2 Bass kernel — 8-core expert parallelism, v2.

Strategy (hardcoded for T=8192, D=1024, F=2048, E=8, top_k=2, 8 cores):
  - Core r owns expert r's w1/w3/w2 (host-precast bf16) and computes the full
    shared expert for its own 1024 tokens (token-sharded shared expert).
  - Router: core r routes its own tokens [1024r, 1024(r+1)) in float32r; the
    renormalized top-2 softmax weights are sigmoid(l1-l2) / 1-sigmoid(l1-l2).
    A small AllToAll ([8,1024] f32) sends each expert's combine-weight column
    to its owner core (out shard a = weights of core a's tokens, my expert).
  - Tokens are split into two halves by position (for split ReduceScatter):
    position p = 4096h + 512r + i  <->  token 1024r + 512h + i. Host permutes
    a bf16 copy of x into that layout (xlo/xhi gather sources).
  - Compaction per half: mask -> cumsum-by-triangular-matmul -> 32 indirect
    scatters of (local position, weight) pairs; then one transposed dma_gather
    per 384-block dispatches token rows and one dma_scatter_add per block
    combines weighted FFN rows into the half's bf16 partial.
  - FFN matmuls keep tokens in the free dim for w1/w3 and use the
    out[t,d] = h[f,t]^T @ w2[f,d] orientation for w2 (no transposes at all).
  - Two ReduceScatters (one per half): the first overlaps second-half compute.
    Shared-expert output (SBUF-resident) is added after RS; out is fp32.
"""
import sys

sys.path.insert(0, "/opt/trn_rl_repo")

import numpy as np
import ml_dtypes

import concourse.bacc as bacc
import concourse.mybir as mybir
import concourse.tile as tile
from concourse.bass import IndirectOffsetOnAxis
from concourse.bass_utils import run_bass_kernel_spmd
from concourse.masks import make_identity

dt = mybir.dt
AF = mybir.ActivationFunctionType
OP = mybir.AluOpType

P = 128
T, D, F, E = 8192, 1024, 2048, 8
TSL = 1024            # own tokens per core
HT = 4096             # tokens per half (global)
C2 = 1152             # expert token capacity per half (max measured 1118)
NCH = 32              # 128-chunks per half
B = 384               # expert FFN block (tokens per gather/scatter)
NB = 3                # blocks per half (3*384 = 1152)
BIG = 1 << 20
RG = [list(range(8))]

_CACHE = {}


def _build():
    if "nc" in _CACHE:
        return _CACHE["nc"]
    nc = bacc.Bacc("TRN2", target_bir_lowering=False, debug=False, num_devices=8)

    xlo_ext = nc.dram_tensor("xlo", [HT, D], dt.bfloat16, kind="ExternalInput")
    xhi_ext = nc.dram_tensor("xhi", [HT, D], dt.bfloat16, kind="ExternalInput")
    xtr_ext = nc.dram_tensor("xtr", [D, TSL], dt.float32, kind="ExternalInput")
    xtb_ext = nc.dram_tensor("xtb", [D, TSL], dt.bfloat16, kind="ExternalInput")
    gw9_ext = nc.dram_tensor("gw9", [D, 16], dt.float32, kind="ExternalInput")
    w1_ext = nc.dram_tensor("w1e", [D, F], dt.bfloat16, kind="ExternalInput")
    w3_ext = nc.dram_tensor("w3e", [D, F], dt.bfloat16, kind="ExternalInput")
    w2_ext = nc.dram_tensor("w2e", [F, D], dt.bfloat16, kind="ExternalInput")
    sw1_ext = nc.dram_tensor("sw1e", [D, F], dt.bfloat16, kind="ExternalInput")
    sw3_ext = nc.dram_tensor("sw3e", [D, F], dt.bfloat16, kind="ExternalInput")
    sw2_ext = nc.dram_tensor("sw2e", [F, D], dt.bfloat16, kind="ExternalInput")
    out_ext = nc.dram_tensor("out", [TSL, D], dt.float32, kind="ExternalOutput")

    with tile.TileContext(nc) as tc:
        with tc.tile_pool(name="cn", bufs=1) as cn, \
             tc.tile_pool(name="wk", bufs=2) as wk, \
             tc.tile_pool(name="ps", bufs=1, space="PSUM") as ps, \
             tc.tile_pool(name="dr", bufs=1, space="DRAM") as dr:

            # ---------------- DRAM scratch ----------------
            a2a_in = dr.tile([8, TSL], dt.float32)
            a2a_out = dr.tile([8, TSL], dt.float32)
            iw_d = [dr.tile([C2, 2], dt.int32, name=f"iw{i}") for i in range(2)]
            pl_d = [dr.tile([HT, D], dt.bfloat16, name=f"pl{i}") for i in range(2)]
            rs_d = [dr.tile([HT // 8, D], dt.bfloat16, name=f"rs{i}")
                    for i in range(2)]

            # ---------------- constants ----------------
            ident_f = cn.tile([NCH, NCH], dt.float32)
            make_identity(nc, ident_f[:])
            ones_bf = cn.tile([P, 1], dt.bfloat16)
            nc.vector.memset(ones_bf[:], 1.0)
            # tri[k, m] = 1 if k < m
            tri_bf = cn.tile([P, P], dt.bfloat16)
            nc.vector.memset(tri_bf[:], 1.0)
            nc.gpsimd.affine_select(
                out=tri_bf[:], in_=tri_bf[:], pattern=[[1, P]], base=-1,
                channel_multiplier=-1, compare_op=OP.is_ge, fill=0.0)
            ones_row_f = cn.tile([1, P], dt.float32)
            nc.vector.memset(ones_row_f[:], 1.0)
            iota8_i = cn.tile([P, E], dt.int32)
            nc.gpsimd.iota(iota8_i[:], pattern=[[1, E]], base=0,
                           channel_multiplier=0)
            iota8_f = cn.tile([P, E], dt.float32)
            nc.vector.tensor_copy(out=iota8_f[:], in_=iota8_i[:])
            # local position ids for compaction chunks: chunk k=a*4+b,
            # id = 512*a + 128*b + p
            iota_h = cn.tile([P, 8, 4], dt.int32)
            nc.gpsimd.iota(iota_h[:], pattern=[[512, 8], [128, 4]], base=0,
                           channel_multiplier=1)

            # ---------------- zero-init partials + iw ----------------
            zb = cn.tile([P, D], dt.bfloat16)
            nc.vector.memset(zb[:], 0.0)
            for h in range(2):
                pr = pl_d[h][:, :].rearrange("(a p) f -> p a f", p=P)
                for g in range(HT // P):
                    nc.sync.dma_start(out=pr[:, g, :], in_=zb[:])
            zi = cn.tile([P, C2 // P, 2], dt.int32)
            nc.vector.memset(zi[:], 0)
            for h in range(2):
                nc.sync.dma_start(
                    out=iw_d[h][:, :].rearrange("(a p) f -> p a f", p=P),
                    in_=zi[:])

            # ---------------- resident weights (host-precast bf16) ----------
            gw9s = cn.tile([P, 8, 16], dt.float32r)
            nc.sync.dma_start(
                out=gw9s[:],
                in_=gw9_ext[:, :].bitcast(dt.float32r)
                .rearrange("(a p) f -> p a f", p=P))
            w1s = cn.tile([P, 8, F], dt.bfloat16)
            w3s = cn.tile([P, 8, F], dt.bfloat16)
            for a in range(8):
                nc.sync.dma_start(out=w1s[:, a, :],
                                  in_=w1_ext[a * P:(a + 1) * P, :])
                nc.sync.dma_start(out=w3s[:, a, :],
                                  in_=w3_ext[a * P:(a + 1) * P, :])
            # big w2 slot: holds sw2 during shared phase, then expert w2
            bigw2 = cn.tile([P, 16, D], dt.bfloat16)
            for a in range(16):
                nc.sync.dma_start(out=bigw2[:, a, :],
                                  in_=sw2_ext[a * P:(a + 1) * P, :])

            # hbuf shared between shared-expert h_s and expert hs
            hbuf = cn.tile([P, 16, 512], dt.bfloat16)
            out_s = cn.tile([P, 8, D], dt.bfloat16)
            payload = cn.tile([P, 8, 9], dt.float32)
            idxs16 = []
            wcol = []

            # ======== phase pool: router + shared expert ========
            with tc.tile_pool(name="sp", bufs=1) as sp:
                xtb = sp.tile([P, 8, TSL], dt.bfloat16, name="xtb")
                for a in range(8):
                    nc.sync.dma_start(out=xtb[:, a, :],
                                      in_=xtb_ext[a * P:(a + 1) * P, :])

                # ---- router on own tokens ----
                for c in range(8):
                    xrt = sp.tile([P, 8, P], dt.float32r, bufs=2, name="xrt")
                    nc.sync.dma_start(
                        out=xrt[:],
                        in_=xtr_ext[:, c * P:(c + 1) * P].bitcast(dt.float32r)
                        .rearrange("(a p) f -> p a f", p=P))
                    lg9 = ps.tile([P, 16], dt.float32, tag="sm", bufs=2,
                                  name="lg9")
                    for a in range(8):
                        nc.tensor.matmul(out=lg9[:], lhsT=xrt[:, a, :],
                                         rhs=gw9s[:, a, :],
                                         start=(a == 0), stop=(a == 7))
                    lgc = sp.tile([P, 9], dt.float32, bufs=2, name="lgc")
                    nc.vector.tensor_copy(out=lgc[:], in_=lg9[:, 0:9])
                    mx = sp.tile([P, 8], dt.float32, bufs=2, name="mx")
                    nc.vector.max(out=mx[:], in_=lgc[:, 0:8])
                    mi = sp.tile([P, 8], dt.uint32, bufs=2, name="mi")
                    nc.vector.max_index(out=mi[:], in_max=mx[:],
                                        in_values=lgc[:, 0:8])
                    mif = sp.tile([P, 2], dt.float32, bufs=2, name="mif")
                    nc.vector.tensor_copy(out=mif[:],
                                          in_=mi[:, 0:2].bitcast(dt.int32))
                    d12 = sp.tile([P, 1], dt.float32, bufs=2, name="d12")
                    nc.vector.tensor_sub(d12[:], mx[:, 0:1], mx[:, 1:2])
                    wA = sp.tile([P, 1], dt.float32, bufs=2, name="wA")
                    nc.scalar.activation(out=wA[:], in_=d12[:], func=AF.Sigmoid)
                    wB = sp.tile([P, 1], dt.float32, bufs=2, name="wB")
                    nc.scalar.activation(out=wB[:], in_=wA[:], func=AF.Copy,
                                         scale=-1.0, bias=1.0)
                    eq1 = sp.tile([P, 8], dt.float32, bufs=2, name="eq1")
                    nc.vector.tensor_tensor(
                        out=eq1[:], in0=mif[:, 0:1].to_broadcast([P, 8]),
                        in1=iota8_f[:], op=OP.is_equal)
                    eq2 = sp.tile([P, 8], dt.float32, bufs=2, name="eq2")
                    nc.vector.tensor_tensor(
                        out=eq2[:], in0=mif[:, 1:2].to_broadcast([P, 8]),
                        in1=iota8_f[:], op=OP.is_equal)
                    nc.vector.tensor_tensor(out=eq1[:], in0=eq1[:],
                                            in1=wA[:].to_broadcast([P, 8]),
                                            op=OP.mult)
                    nc.vector.tensor_tensor(out=eq2[:], in0=eq2[:],
                                            in1=wB[:].to_broadcast([P, 8]),
                                            op=OP.mult)
                    nc.vector.tensor_add(payload[:, c, 0:8], eq1[:], eq2[:])
                    nc.scalar.activation(out=payload[:, c, 8:9],
                                         in_=lgc[:, 8:9], func=AF.Sigmoid)
                # AllToAll: shard e = weight column e for my 1024 tokens; out
                # shard a = weights of core a's tokens for MY expert.
                for e in range(8):
                    nc.sync.dma_start(
                        out=a2a_in[e:e + 1, :].rearrange("o (c p) -> p (o c)",
                                                         p=P),
                        in_=payload[:, :, e])
                nc.gpsimd.collective_compute(
                    "AllToAll", OP.bypass, replica_groups=RG,
                    ins=[a2a_in[:, :].opt()], outs=[a2a_out[:, :].opt()])

                def shared_half(h):
                    # h_s[f, t] for own tokens [512h, 512h+512)
                    t0 = 512 * h
                    for fk in range(16):
                        s1t = sp.tile([P, 8, P], dt.bfloat16, bufs=2,
                                      name="s1t")
                        nc.sync.dma_start(
                            out=s1t[:],
                            in_=sw1_ext[:, fk * P:(fk + 1) * P]
                            .rearrange("(a p) f -> p a f", p=P))
                        s3t = sp.tile([P, 8, P], dt.bfloat16, bufs=2,
                                      name="s3t")
                        nc.sync.dma_start(
                            out=s3t[:],
                            in_=sw3_ext[:, fk * P:(fk + 1) * P]
                            .rearrange("(a p) f -> p a f", p=P))
                        ph1 = ps.tile([P, 512], dt.float32, tag="mm", bufs=3,
                                      name="ph1")
                        for a in range(8):
                            nc.tensor.matmul(out=ph1[:], lhsT=s1t[:, a, :],
                                             rhs=xtb[:, a, t0:t0 + 512],
                                             start=(a == 0), stop=(a == 7))
                        ph3 = ps.tile([P, 512], dt.float32, tag="mm", bufs=3,
                                      name="ph3")
                        for a in range(8):
                            nc.tensor.matmul(out=ph3[:], lhsT=s3t[:, a, :],
                                             rhs=xtb[:, a, t0:t0 + 512],
                                             start=(a == 0), stop=(a == 7))
                        hg = sp.tile([P, 512], dt.bfloat16, bufs=2, name="hg")
                        nc.scalar.activation(out=hg[:], in_=ph1[:],
                                             func=AF.Silu)
                        h3b = sp.tile([P, 512], dt.bfloat16, bufs=2,
                                      name="h3b")
                        nc.scalar.activation(out=h3b[:], in_=ph3[:],
                                             func=AF.Copy)
                        nc.vector.tensor_mul(hbuf[:, fk, 0:512], hg[:],
                                             h3b[:])

                def shared_half_w2(h):
                    for tc_ in range(4):
                        for dh in range(2):
                            po = ps.tile([P, 512], dt.float32, tag="mm",
                                         bufs=3, name="po")
                            for fk in range(16):
                                nc.tensor.matmul(
                                    out=po[:],
                                    lhsT=hbuf[:, fk, tc_ * P:(tc_ + 1) * P],
                                    rhs=bigw2[:, fk, dh * 512:(dh + 1) * 512],
                                    start=(fk == 0), stop=(fk == 15))
                            nc.vector.tensor_scalar_mul(
                                out_s[:, 4 * h + tc_, dh * 512:(dh + 1) * 512],
                                po[:], payload[:, 4 * h + tc_, 8:9])

                # ---- shared expert half 0 (w1/w3) ----
                shared_half(0)

                # ---- compaction (both halves) ----
                for h in range(2):
                    # cwe[:, a*4+b] = combine weight of owner-order row
                    # 1024a + 512h + 128b + p for this core's expert
                    cwe = cn.tile([P, NCH], dt.float32, name=f"cwe{h}")
                    for a in range(8):
                        nc.sync.dma_start(
                            out=cwe[:, 4 * a:4 * (a + 1)],
                            in_=a2a_out[a:a + 1, 512 * h:512 * h + 512]
                            .rearrange("o (c p) -> p (o c)", p=P))
                    mask_f = cn.tile([P, NCH], dt.float32, name=f"maskf{h}")
                    nc.vector.tensor_scalar(out=mask_f[:], in0=cwe[:],
                                            scalar1=0.0, scalar2=None,
                                            op0=OP.is_gt)
                    mask_bf = cn.tile([P, NCH], dt.bfloat16, name=f"maskb{h}")
                    nc.vector.tensor_copy(out=mask_bf[:], in_=mask_f[:])

                    # column sums -> exclusive prefix over 32 columns
                    pcst = ps.tile([P, 1], dt.float32, tag="sm", bufs=2,
                                   name="pcst")
                    nc.tensor.matmul(out=pcst[0:NCH, :], lhsT=mask_bf[:],
                                     rhs=ones_bf[:], start=True, stop=True)
                    cst = wk.tile([NCH, 1], dt.bfloat16, bufs=2, name="cst")
                    nc.vector.tensor_copy(out=cst[:], in_=pcst[0:NCH, :])
                    ppre = ps.tile([P, 1], dt.float32, tag="sm", bufs=2,
                                   name="ppre")
                    nc.tensor.matmul(out=ppre[0:NCH, :],
                                     lhsT=tri_bf[0:NCH, 0:NCH], rhs=cst[:],
                                     start=True, stop=True)
                    pre_sb = wk.tile([NCH, 1], dt.float32, bufs=2,
                                     name="pre_sb")
                    nc.vector.tensor_copy(out=pre_sb[:], in_=ppre[0:NCH, :])
                    pprer = ps.tile([1, NCH], dt.float32, tag="sm", bufs=2,
                                    name="pprer")
                    nc.tensor.transpose(out=pprer[:], in_=pre_sb[:],
                                        identity=ident_f[:])
                    pre_row = wk.tile([1, NCH], dt.float32, bufs=2,
                                      name="pre_row")
                    nc.vector.tensor_copy(out=pre_row[:], in_=pprer[:])

                    ppos = ps.tile([P, NCH], dt.float32, tag="sm", bufs=2,
                                   name="ppos")
                    nc.tensor.matmul(out=ppos[:], lhsT=tri_bf[:],
                                     rhs=mask_bf[:], start=True, stop=False)
                    nc.tensor.matmul(out=ppos[:], lhsT=ones_row_f[:],
                                     rhs=pre_row[:], start=False, stop=True)
                    posm = wk.tile([P, NCH], dt.float32, bufs=2, name="posm")
                    nc.vector.tensor_tensor(out=posm[:], in0=ppos[:],
                                            in1=mask_f[:], op=OP.mult)
                    dump = wk.tile([P, NCH], dt.float32, bufs=2, name="dump")
                    nc.vector.tensor_scalar(out=dump[:], in0=mask_f[:],
                                            scalar1=float(-BIG),
                                            scalar2=float(BIG),
                                            op0=OP.mult, op1=OP.add)
                    nc.vector.tensor_add(posm[:], posm[:], dump[:])
                    o_i = cn.tile([P, NCH], dt.int32, name=f"oi{h}")
                    nc.vector.tensor_copy(out=o_i[:], in_=posm[:])

                    iw_pack = cn.tile([P, NCH, 2], dt.int32, name=f"iwp{h}")
                    for a in range(8):
                        nc.vector.tensor_copy(
                            out=iw_pack[:, 4 * a:4 * (a + 1), 0],
                            in_=iota_h[:, a, :])
                    nc.vector.tensor_copy(out=iw_pack[:, :, 1],
                                          in_=cwe[:].bitcast(dt.int32))
                    for k in range(NCH):
                        nc.gpsimd.indirect_dma_start(
                            out=iw_d[h][:, :],
                            out_offset=IndirectOffsetOnAxis(ap=o_i[:, k:k + 1],
                                                            axis=0),
                            in_=iw_pack[:, k, :], in_offset=None,
                            bounds_check=C2 - 1, oob_is_err=False)

                    idx = cn.tile([P, C2 // 16], dt.int16, name=f"idx{h}")
                    for grp in range(8):
                        nc.sync.dma_start(
                            out=idx[grp * 16:(grp + 1) * 16, :],
                            in_=iw_d[h][:, :].bitcast(dt.int16)[:, 0:1]
                            .rearrange("(s p) f -> p (s f)", p=16))
                    idxs16.append(idx)
                    wc = cn.tile([P, C2 // P], dt.float32, name=f"wc{h}")
                    nc.sync.dma_start(
                        out=wc[:],
                        in_=iw_d[h][:, :].bitcast(dt.float32)[:, 1:2]
                        .rearrange("(c p) f -> p (c f)", p=P))
                    wcol.append(wc)

                # ---- shared expert: finish ----
                shared_half_w2(0)
                shared_half(1)
                shared_half_w2(1)

            # ======== phase pool: expert FFN + output ========
            with tc.tile_pool(name="ep", bufs=1) as ep:
                # expert w2 into the big slot (replaces sw2)
                for a in range(16):
                    nc.sync.dma_start(out=bigw2[:, a, :],
                                      in_=w2_ext[a * P:(a + 1) * P, :])

                def emit_gathers(h, xsrc):
                    xgs = []
                    for b in range(NB):
                        xg = ep.tile([P, 8, B], dt.bfloat16, bufs=3, name="xg")
                        nc.gpsimd.dma_gather(
                            xg[:], xsrc[:, :],
                            idxs16[h][:, 24 * b:24 * (b + 1)],
                            B, B, D, transpose=True)
                        xgs.append(xg)
                    return xgs

                def emit_scatters(h, obs):
                    for b in range(NB):
                        nc.gpsimd.dma_scatter_add(
                            pl_d[h][:, :], obs[b][:],
                            idxs16[h][:, 24 * b:24 * (b + 1)], B, B, D)

                def expert_compute(h, xgs):
                    obs = []
                    for b in range(NB):
                        xg = xgs[b]
                        for fk in range(16):
                            ph1 = ps.tile([P, 512], dt.float32, tag="mm",
                                          bufs=3, name="ph1")
                            for a in range(8):
                                nc.tensor.matmul(
                                    out=ph1[:, 0:B],
                                    lhsT=w1s[:, a, fk * P:(fk + 1) * P],
                                    rhs=xg[:, a, :], start=(a == 0),
                                    stop=(a == 7))
                            ph3 = ps.tile([P, 512], dt.float32, tag="mm",
                                          bufs=3, name="ph3")
                            for a in range(8):
                                nc.tensor.matmul(
                                    out=ph3[:, 0:B],
                                    lhsT=w3s[:, a, fk * P:(fk + 1) * P],
                                    rhs=xg[:, a, :], start=(a == 0),
                                    stop=(a == 7))
                            hg = ep.tile([P, B], dt.bfloat16, bufs=2,
                                         name="ehg")
                            nc.scalar.activation(out=hg[:], in_=ph1[:, 0:B],
                                                 func=AF.Silu)
                            h3b = ep.tile([P, B], dt.bfloat16, bufs=2,
                                          name="eh3b")
                            nc.scalar.activation(out=h3b[:], in_=ph3[:, 0:B],
                                                 func=AF.Copy)
                            nc.vector.tensor_mul(hbuf[:, fk, 0:B], hg[:],
                                                 h3b[:])
                        ob = ep.tile([P, NB, D], dt.bfloat16, bufs=3,
                                     name="ob")
                        for tc_ in range(NB):
                            for dh in range(2):
                                po = ps.tile([P, 512], dt.float32, tag="mm",
                                             bufs=3, name="po")
                                for fk in range(16):
                                    nc.tensor.matmul(
                                        out=po[:],
                                        lhsT=hbuf[:, fk, tc_ * P:(tc_ + 1) * P],
                                        rhs=bigw2[:, fk,
                                                  dh * 512:(dh + 1) * 512],
                                        start=(fk == 0), stop=(fk == 15))
                                nc.vector.tensor_scalar_mul(
                                    ob[:, tc_, dh * 512:(dh + 1) * 512], po[:],
                                    wcol[h][:, 3 * b + tc_:3 * b + tc_ + 1])
                        obs.append(ob)
                    return obs

                xg0 = emit_gathers(0, xlo_ext)
                obs0 = expert_compute(0, xg0)
                # h1 gathers after h0 compute (xg slot WAR) but before the h0
                # scatters, so they issue on gpsimd during h0's tail compute.
                xg1 = emit_gathers(1, xhi_ext)
                emit_scatters(0, obs0)
                # RS on half 0 overlaps half-1 compute (gpsimd queue: after h0
                # scatters, before h1 scatters).
                nc.gpsimd.collective_compute(
                    "ReduceScatter", OP.add, replica_groups=RG,
                    ins=[pl_d[0][:, :].opt()], outs=[rs_d[0][:, :].opt()])
                obs1 = expert_compute(1, xg1)
                emit_scatters(1, obs1)

                def emit_out(h):
                    rsl = ep.tile([P, 4, D], dt.bfloat16, bufs=1, name="rsl")
                    nc.sync.dma_start(
                        out=rsl[:],
                        in_=rs_d[h][:, :].rearrange("(c p) f -> p c f", p=P))
                    for tc_ in range(4):
                        of = ep.tile([P, D], dt.float32, bufs=1, name="of")
                        nc.vector.tensor_add(of[:], rsl[:, tc_, :],
                                             out_s[:, 4 * h + tc_, :])
                        nc.sync.dma_start(
                            out=out_ext[:, :].rearrange("(c p) f -> p c f",
                                                        p=P)[:, 4 * h + tc_, :],
                            in_=of[:])

                emit_out(0)
                nc.gpsimd.collective_compute(
                    "ReduceScatter", OP.add, replica_groups=RG,
                    ins=[pl_d[1][:, :].opt()], outs=[rs_d[1][:, :].opt()])
                emit_out(1)

    nc.compile()
    _CACHE["nc"] = nc
    return nc


def _shard(inputs):
    bf16 = ml_dtypes.bfloat16
    x = np.ascontiguousarray(np.asarray(inputs["hidden_states"], np.float32))
    xbf = x.astype(bf16)
    # position p = 4096h + 512r + i  <->  token 1024r + 512h + i
    xperm = np.ascontiguousarray(
        xbf.reshape(8, 2, 512, D).transpose(1, 0, 2, 3).reshape(2, HT, D))
    gw9 = np.zeros((D, 16), np.float32)
    gw9[:, 0:8] = np.asarray(inputs["gate_w"], np.float32)
    gw9[:, 8:9] = np.asarray(inputs["sgate_w"], np.float32)
    w1 = np.asarray(inputs["w1"], np.float32).astype(bf16)
    w3 = np.asarray(inputs["w3"], np.float32).astype(bf16)
    w2 = np.asarray(inputs["w2"], np.float32).astype(bf16)
    sw1 = np.ascontiguousarray(np.asarray(inputs["sw1"], np.float32).astype(bf16))
    sw3 = np.ascontiguousarray(np.asarray(inputs["sw3"], np.float32).astype(bf16))
    sw2 = np.ascontiguousarray(np.asarray(inputs["sw2"], np.float32).astype(bf16))
    in_maps = []
    for r in range(8):
        own = slice(1024 * r, 1024 * (r + 1))
        in_maps.append(dict(
            xlo=xperm[0],
            xhi=xperm[1],
            xtr=np.ascontiguousarray(x[own].T),
            xtb=np.ascontiguousarray(xbf[own].T),
            gw9=gw9,
            w1e=np.ascontiguousarray(w1[r]),
            w3e=np.ascontiguousarray(w3[r]),
            w2e=np.ascontiguousarray(w2[r]),
            sw1e=sw1,
            sw3e=sw3,
            sw2e=sw2,
        ))
    return in_maps


def run(inputs, trace=False):
    nc = _build()
    in_maps = _shard(inputs)
    res = run_bass_kernel_spmd(nc, in_maps, list(range(8)), trace=trace)
    out = np.concatenate([res.results[r]["out"] for r in range(8)], axis=0)
    return out.astype(np.float32), res


def kernel(**inputs):
    out, _ = run(inputs, trace=False)
    return out


# revision 20
# speedup vs baseline: 1.5257x; 1.0339x over previous
"""MoE layer (moe_routing) Trainium2 Bass kernel — 8-core expert parallelism, v6.

Strategy (hardcoded for T=8192, D=1024, F=2048, E=8, top_k=2, 8 cores):
  - Core r owns expert r's w1/w3/w2 (host-precast bf16) and computes the full
    shared expert for its own 1024 tokens (token-sharded shared expert).
  - Router: core r routes its own tokens [1024r, 1024(r+1)) in float32r; the
    renormalized top-2 softmax weights are sigmoid(l1-l2) / 1-sigmoid(l1-l2).
    A small AllToAll ([8,1024] f32) sends each expert's combine-weight column
    to its owner core (out shard a = weights of core a's tokens, my expert).
  - Tokens are split into two halves by position (for split ReduceScatter):
    position p = 4096h + 512r + i  <->  token 1024r + 512h + i. Host permutes
    a bf16 copy of x into that layout (xlo/xhi gather sources).
  - Compaction per half: mask -> cumsum-by-triangular-matmul -> 32 indirect
    scatters of (local position, weight) pairs (the two halves' scatter
    chains are interleaved so their DMA round-trips overlap); then one
    transposed dma_gather per 384-block dispatches token rows and one
    dma_scatter_add per block combines weighted FFN rows into the half's
    bf16 partial.
  - FFN matmuls keep tokens in the free dim for w1/w3 and use the
    out[t,d] = h[f,t]^T @ w2[f,d] orientation for w2 (no transposes at all).
  - Two ReduceScatters (one per half): the first overlaps half-1 compute.
    Shared-expert output (SBUF-resident) is added after RS on gpsimd (so the
    RS wait cannot block the DVE pipeline); out is fp32.
  - Emission order tuned so the DMA queues serve the router/shared streams
    first; expert-weight loads are spread across the second-half stream
    slots; partial zero-fill rides an idle DMA window late.
"""
import sys

sys.path.insert(0, "/opt/trn_rl_repo")

import numpy as np
import ml_dtypes

import concourse.bacc as bacc
import concourse.mybir as mybir
import concourse.tile as tile
from concourse.bass import IndirectOffsetOnAxis
from concourse.bass_utils import run_bass_kernel_spmd
from concourse.masks import make_identity

dt = mybir.dt
AF = mybir.ActivationFunctionType
OP = mybir.AluOpType

P = 128
T, D, F, E = 8192, 1024, 2048, 8
TSL = 1024            # own tokens per core
HT = 4096             # tokens per half (global)
C2 = 1152             # expert token capacity per half (max measured 1118)
NCH = 32              # 128-chunks per half
B = 384               # expert FFN block (tokens per gather/scatter)
NB = 3                # blocks per half (3*384 = 1152)
BIG = 1 << 20
RG = [list(range(8))]

_CACHE = {}


def _build():
    if "nc" in _CACHE:
        return _CACHE["nc"]
    nc = bacc.Bacc("TRN2", target_bir_lowering=False, debug=False, num_devices=8)

    xlo_ext = nc.dram_tensor("xlo", [HT, D], dt.bfloat16, kind="ExternalInput")
    xhi_ext = nc.dram_tensor("xhi", [HT, D], dt.bfloat16, kind="ExternalInput")
    xtr_ext = nc.dram_tensor("xtr", [D, TSL], dt.float32, kind="ExternalInput")
    xtb_ext = nc.dram_tensor("xtb", [D, TSL], dt.bfloat16, kind="ExternalInput")
    gw9_ext = nc.dram_tensor("gw9", [D, 16], dt.float32, kind="ExternalInput")
    w1_ext = nc.dram_tensor("w1e", [D, F], dt.bfloat16, kind="ExternalInput")
    w3_ext = nc.dram_tensor("w3e", [D, F], dt.bfloat16, kind="ExternalInput")
    w2_ext = nc.dram_tensor("w2e", [F, D], dt.bfloat16, kind="ExternalInput")
    sw1_ext = nc.dram_tensor("sw1e", [D, F], dt.bfloat16, kind="ExternalInput")
    sw3_ext = nc.dram_tensor("sw3e", [D, F], dt.bfloat16, kind="ExternalInput")
    sw2_ext = nc.dram_tensor("sw2e", [F, D], dt.bfloat16, kind="ExternalInput")
    out_ext = nc.dram_tensor("out", [TSL, D], dt.float32, kind="ExternalOutput")

    with tile.TileContext(nc) as tc:
        with tc.tile_pool(name="cn", bufs=1) as cn, \
             tc.tile_pool(name="wk", bufs=2) as wk, \
             tc.tile_pool(name="ps", bufs=1, space="PSUM") as ps, \
             tc.tile_pool(name="dr", bufs=1, space="DRAM") as dr:

            # ---------------- DRAM scratch ----------------
            a2a_in = dr.tile([8, TSL], dt.float32)
            a2a_out = dr.tile([8, TSL], dt.float32)
            iw_d = [dr.tile([C2, 2], dt.int32, name=f"iw{i}") for i in range(2)]
            pl_d = [dr.tile([HT, D], dt.bfloat16, name=f"pl{i}") for i in range(2)]
            rs_d = [dr.tile([HT // 8, D], dt.bfloat16, name=f"rs{i}")
                    for i in range(2)]

            # ---------------- constants ----------------
            ident_f = cn.tile([NCH, NCH], dt.float32)
            make_identity(nc, ident_f[:])
            tri_bf = cn.tile([P, P], dt.bfloat16)
            nc.vector.memset(tri_bf[:], 1.0)
            nc.gpsimd.affine_select(
                out=tri_bf[:], in_=tri_bf[:], pattern=[[1, P]], base=-1,
                channel_multiplier=-1, compare_op=OP.is_ge, fill=0.0)
            ones_row_f = cn.tile([1, P], dt.float32)
            nc.vector.memset(ones_row_f[:], 1.0)
            iota8_i = cn.tile([P, E], dt.int32)
            nc.gpsimd.iota(iota8_i[:], pattern=[[1, E]], base=0,
                           channel_multiplier=0)
            iota8_f = cn.tile([P, E], dt.float32)
            nc.vector.tensor_copy(out=iota8_f[:], in_=iota8_i[:])
            # local position ids: chunk k=a*4+b holds ids 512a+128b+p
            iota_h = cn.tile([P, 8, 4], dt.int32)
            nc.gpsimd.iota(iota_h[:], pattern=[[512, 8], [128, 4]], base=0,
                           channel_multiplier=1)

            # iw zero-init (tiny, early; pl zero-fill is emitted late)
            zi = cn.tile([P, C2 // P, 2], dt.int32)
            nc.vector.memset(zi[:], 0)
            for h in range(2):
                nc.sync.dma_start(
                    out=iw_d[h][:, :].rearrange("(a p) f -> p a f", p=P),
                    in_=zi[:])

            gw9s = cn.tile([P, 8, 16], dt.float32r)
            nc.sync.dma_start(
                out=gw9s[:],
                in_=gw9_ext[:, :].bitcast(dt.float32r)
                .rearrange("(a p) f -> p a f", p=P))

            w1s = cn.tile([P, 8, F], dt.bfloat16)
            w3s = cn.tile([P, 8, F], dt.bfloat16)
            w2s = cn.tile([P, 16, D], dt.bfloat16)
            hbuf = cn.tile([P, 16, 512], dt.bfloat16)
            out_s = cn.tile([P, 8, D], dt.bfloat16)
            payload = cn.tile([P, 8, 9], dt.float32)
            idxs16 = []
            wcol = []

            # ======== phase pool: router + shared expert ========
            with tc.tile_pool(name="sp", bufs=1) as sp:
                xtb = sp.tile([P, 8, TSL], dt.bfloat16, name="xtb")
                for a in range(8):
                    nc.sync.dma_start(out=xtb[:, a, :],
                                      in_=xtb_ext[a * P:(a + 1) * P, :])

                # ---- router on own tokens ----
                for c in range(8):
                    xrt = sp.tile([P, 8, P], dt.float32r, bufs=2, name="xrt")
                    nc.sync.dma_start(
                        out=xrt[:],
                        in_=xtr_ext[:, c * P:(c + 1) * P].bitcast(dt.float32r)
                        .rearrange("(a p) f -> p a f", p=P))
                    lg9 = ps.tile([P, 16], dt.float32, tag="sm", bufs=1,
                                  name="lg9")
                    for a in range(8):
                        nc.tensor.matmul(out=lg9[:], lhsT=xrt[:, a, :],
                                         rhs=gw9s[:, a, :],
                                         start=(a == 0), stop=(a == 7))
                    lgc = sp.tile([P, 9], dt.float32, bufs=2, name="lgc")
                    nc.vector.tensor_copy(out=lgc[:], in_=lg9[:, 0:9])
                    mx = sp.tile([P, 8], dt.float32, bufs=2, name="mx")
                    nc.vector.max(out=mx[:], in_=lgc[:, 0:8])
                    mi = sp.tile([P, 8], dt.uint32, bufs=2, name="mi")
                    nc.vector.max_index(out=mi[:], in_max=mx[:],
                                        in_values=lgc[:, 0:8])
                    mif = sp.tile([P, 2], dt.float32, bufs=2, name="mif")
                    nc.vector.tensor_copy(out=mif[:],
                                          in_=mi[:, 0:2].bitcast(dt.int32))
                    d12 = sp.tile([P, 1], dt.float32, bufs=2, name="d12")
                    nc.vector.tensor_sub(d12[:], mx[:, 0:1], mx[:, 1:2])
                    wA = sp.tile([P, 1], dt.float32, bufs=2, name="wA")
                    nc.scalar.activation(out=wA[:], in_=d12[:], func=AF.Sigmoid)
                    wB = sp.tile([P, 1], dt.float32, bufs=2, name="wB")
                    nc.scalar.activation(out=wB[:], in_=wA[:], func=AF.Copy,
                                         scale=-1.0, bias=1.0)
                    eq1 = sp.tile([P, 8], dt.float32, bufs=2, name="eq1")
                    nc.vector.tensor_tensor(
                        out=eq1[:], in0=mif[:, 0:1].to_broadcast([P, 8]),
                        in1=iota8_f[:], op=OP.is_equal)
                    eq2 = sp.tile([P, 8], dt.float32, bufs=2, name="eq2")
                    nc.vector.tensor_tensor(
                        out=eq2[:], in0=mif[:, 1:2].to_broadcast([P, 8]),
                        in1=iota8_f[:], op=OP.is_equal)
                    nc.vector.tensor_tensor(out=eq1[:], in0=eq1[:],
                                            in1=wA[:].to_broadcast([P, 8]),
                                            op=OP.mult)
                    nc.vector.tensor_tensor(out=eq2[:], in0=eq2[:],
                                            in1=wB[:].to_broadcast([P, 8]),
                                            op=OP.mult)
                    nc.vector.tensor_add(payload[:, c, 0:8], eq1[:], eq2[:])
                    nc.scalar.activation(out=payload[:, c, 8:9],
                                         in_=lgc[:, 8:9], func=AF.Sigmoid)

                def shared_w13(h):
                    # h_s[f, t] for own tokens [512h, 512h+512); expert w1/w3
                    # resident loads are spread across the h1 stream slots
                    t0 = 512 * h
                    for fk in range(16):
                        s1t = sp.tile([P, 8, P], dt.bfloat16, bufs=3,
                                      name="s1t")
                        nc.sync.dma_start(
                            out=s1t[:],
                            in_=sw1_ext[:, fk * P:(fk + 1) * P]
                            .rearrange("(a p) f -> p a f", p=P))
                        s3t = sp.tile([P, 8, P], dt.bfloat16, bufs=3,
                                      name="s3t")
                        nc.sync.dma_start(
                            out=s3t[:],
                            in_=sw3_ext[:, fk * P:(fk + 1) * P]
                            .rearrange("(a p) f -> p a f", p=P))
                        if h == 1:
                            if fk < 8:
                                nc.sync.dma_start(
                                    out=w1s[:, fk, :],
                                    in_=w1_ext[fk * P:(fk + 1) * P, :])
                            else:
                                a2 = fk - 8
                                nc.sync.dma_start(
                                    out=w3s[:, a2, :],
                                    in_=w3_ext[a2 * P:(a2 + 1) * P, :])
                        ph1 = ps.tile([P, 512], dt.float32, tag="mm", bufs=3,
                                      name="ph1")
                        for a in range(8):
                            nc.tensor.matmul(out=ph1[:], lhsT=s1t[:, a, :],
                                             rhs=xtb[:, a, t0:t0 + 512],
                                             start=(a == 0), stop=(a == 7))
                        ph3 = ps.tile([P, 512], dt.float32, tag="mm", bufs=3,
                                      name="ph3")
                        for a in range(8):
                            nc.tensor.matmul(out=ph3[:], lhsT=s3t[:, a, :],
                                             rhs=xtb[:, a, t0:t0 + 512],
                                             start=(a == 0), stop=(a == 7))
                        hg = sp.tile([P, 512], dt.bfloat16, bufs=2, name="hg")
                        nc.scalar.activation(out=hg[:], in_=ph1[:],
                                             func=AF.Silu)
                        h3b = sp.tile([P, 512], dt.bfloat16, bufs=2,
                                      name="h3b")
                        nc.scalar.activation(out=h3b[:], in_=ph3[:],
                                             func=AF.Copy)
                        nc.vector.tensor_mul(hbuf[:, fk, 0:512], hg[:],
                                             h3b[:])

                def shared_w2x(h):
                    # stream sw2; expert w2 resident loads spread across the
                    # h1 slots; 4 token-chunk accumulators live per dh
                    for dh in range(2):
                        pos = [ps.tile([P, 512], dt.float32, tag="po4",
                                       bufs=4, name="pos")
                               for _ in range(4)]
                        for fk in range(16):
                            s2t = wk.tile([P, 512], dt.bfloat16, bufs=3,
                                          name="s2t")
                            nc.sync.dma_start(
                                out=s2t[:],
                                in_=sw2_ext[fk * P:(fk + 1) * P,
                                            dh * 512:(dh + 1) * 512])
                            if h == 1 and fk % 2 == 0:
                                a2 = dh * 8 + fk // 2
                                nc.sync.dma_start(
                                    out=w2s[:, a2, :],
                                    in_=w2_ext[a2 * P:(a2 + 1) * P, :])
                            for tc_ in range(4):
                                nc.tensor.matmul(
                                    out=pos[tc_][:],
                                    lhsT=hbuf[:, fk, tc_ * P:(tc_ + 1) * P],
                                    rhs=s2t[:],
                                    start=(fk == 0), stop=(fk == 15))
                        for tc_ in range(4):
                            nc.vector.tensor_scalar_mul(
                                out_s[:, 4 * h + tc_, dh * 512:(dh + 1) * 512],
                                pos[tc_][:], payload[:, 4 * h + tc_, 8:9])

                shared_w13(0)
                shared_w2x(0)
                shared_w13(1)

            # ======== phase pool: compaction + expert FFN + output ========
            with tc.tile_pool(name="ep", bufs=1) as ep:
                # AllToAll (extraction + collective + cwe loads on gpsimd)
                for e in range(8):
                    nc.gpsimd.dma_start(
                        out=a2a_in[e:e + 1, :].rearrange("o (c p) -> p (o c)",
                                                         p=P),
                        in_=payload[:, :, e])
                nc.gpsimd.collective_compute(
                    "AllToAll", OP.bypass, replica_groups=RG,
                    ins=[a2a_in[:, :].opt()], outs=[a2a_out[:, :].opt()])
                cwes = []
                for h in range(2):
                    cwe = cn.tile([P, NCH], dt.float32, name=f"cwe{h}")
                    for a in range(8):
                        nc.gpsimd.dma_start(
                            out=cwe[:, 4 * a:4 * (a + 1)],
                            in_=a2a_out[a:a + 1, 512 * h:512 * h + 512]
                            .rearrange("o (c p) -> p (o c)", p=P))
                    cwes.append(cwe)

                # anchor: comp matmuls use ones_late (produced from the first
                # shared-w2 evac) so the scheduler cannot place them in the
                # tensor stream before the shared w2 stage has begun (the
                # real A2A latency would stall the PE there).
                ones_late = cn.tile([P, 1], dt.bfloat16)
                nc.vector.tensor_scalar(out=ones_late[:],
                                        in0=out_s[:, 0, 0:1], scalar1=0.0,
                                        scalar2=1.0, op0=OP.mult, op1=OP.add)

                def compact_prep(h):
                    cwe = cwes[h]
                    mask_f = cn.tile([P, NCH], dt.float32, name=f"maskf{h}")
                    nc.vector.tensor_scalar(out=mask_f[:], in0=cwe[:],
                                            scalar1=0.0, scalar2=None,
                                            op0=OP.is_gt)
                    mask_bf = cn.tile([P, NCH], dt.bfloat16, name=f"maskb{h}")
                    nc.vector.tensor_copy(out=mask_bf[:], in_=mask_f[:])

                    pcst = ps.tile([P, 1], dt.float32, tag="sm", bufs=1,
                                   name="pcst")
                    nc.tensor.matmul(out=pcst[0:NCH, :], lhsT=mask_bf[:],
                                     rhs=ones_late[:], start=True, stop=True)
                    cst = wk.tile([NCH, 1], dt.bfloat16, bufs=2, name="cst")
                    nc.vector.tensor_copy(out=cst[:], in_=pcst[0:NCH, :])
                    ppre = ps.tile([P, 1], dt.float32, tag="sm", bufs=1,
                                   name="ppre")
                    nc.tensor.matmul(out=ppre[0:NCH, :],
                                     lhsT=tri_bf[0:NCH, 0:NCH], rhs=cst[:],
                                     start=True, stop=True)
                    pre_sb = wk.tile([NCH, 1], dt.float32, bufs=2,
                                     name="pre_sb")
                    nc.vector.tensor_copy(out=pre_sb[:], in_=ppre[0:NCH, :])
                    pprer = ps.tile([1, NCH], dt.float32, tag="sm", bufs=1,
                                    name="pprer")
                    nc.tensor.transpose(out=pprer[:], in_=pre_sb[:],
                                        identity=ident_f[:])
                    pre_row = wk.tile([1, NCH], dt.float32, bufs=2,
                                      name="pre_row")
                    nc.vector.tensor_copy(out=pre_row[:], in_=pprer[:])

                    ppos = ps.tile([P, NCH], dt.float32, tag="sm", bufs=1,
                                   name="ppos")
                    nc.tensor.matmul(out=ppos[:], lhsT=tri_bf[:],
                                     rhs=mask_bf[:], start=True, stop=False)
                    nc.tensor.matmul(out=ppos[:], lhsT=ones_row_f[:],
                                     rhs=pre_row[:], start=False, stop=True)
                    posm = wk.tile([P, NCH], dt.float32, bufs=2, name="posm")
                    nc.vector.tensor_tensor(out=posm[:], in0=ppos[:],
                                            in1=mask_f[:], op=OP.mult)
                    dump = wk.tile([P, NCH], dt.float32, bufs=2, name="dump")
                    nc.vector.tensor_scalar(out=dump[:], in0=mask_f[:],
                                            scalar1=float(-BIG),
                                            scalar2=float(BIG),
                                            op0=OP.mult, op1=OP.add)
                    nc.vector.tensor_add(posm[:], posm[:], dump[:])
                    o_i = cn.tile([P, NCH], dt.int32, name=f"oi{h}")
                    nc.vector.tensor_copy(out=o_i[:], in_=posm[:])

                    iw_pack = cn.tile([P, NCH, 2], dt.int32, name=f"iwp{h}")
                    for a in range(8):
                        nc.vector.tensor_copy(
                            out=iw_pack[:, 4 * a:4 * (a + 1), 0],
                            in_=iota_h[:, a, :])
                    nc.vector.tensor_copy(out=iw_pack[:, :, 1],
                                          in_=cwe[:].bitcast(dt.int32))
                    return o_i, iw_pack

                oi0, iwp0 = compact_prep(0)
                oi1, iwp1 = compact_prep(1)
                # interleave the halves' scatters: two independent WAW chains
                for k in range(NCH):
                    nc.gpsimd.indirect_dma_start(
                        out=iw_d[0][:, :],
                        out_offset=IndirectOffsetOnAxis(ap=oi0[:, k:k + 1],
                                                        axis=0),
                        in_=iwp0[:, k, :], in_offset=None,
                        bounds_check=C2 - 1, oob_is_err=False)
                    nc.gpsimd.indirect_dma_start(
                        out=iw_d[1][:, :],
                        out_offset=IndirectOffsetOnAxis(ap=oi1[:, k:k + 1],
                                                        axis=0),
                        in_=iwp1[:, k, :], in_offset=None,
                        bounds_check=C2 - 1, oob_is_err=False)

                for h in range(2):
                    idx = cn.tile([P, C2 // 16], dt.int16, name=f"idx{h}")
                    for grp in range(8):
                        nc.gpsimd.dma_start(
                            out=idx[grp * 16:(grp + 1) * 16, :],
                            in_=iw_d[h][:, :].bitcast(dt.int16)[:, 0:1]
                            .rearrange("(s p) f -> p (s f)", p=16))
                    idxs16.append(idx)
                    wc = cn.tile([P, C2 // P], dt.float32, name=f"wc{h}")
                    nc.gpsimd.dma_start(
                        out=wc[:],
                        in_=iw_d[h][:, :].bitcast(dt.float32)[:, 1:2]
                        .rearrange("(c p) f -> p (c f)", p=P))
                    wcol.append(wc)

                def emit_gathers(h, xsrc, name):
                    xgs = []
                    for b in range(NB):
                        xg = ep.tile([P, 8, B], dt.bfloat16, bufs=2, name=name)
                        nc.gpsimd.dma_gather(
                            xg[:], xsrc[:, :],
                            idxs16[h][:, 24 * b:24 * (b + 1)],
                            B, B, D, transpose=True)
                        xgs.append(xg)
                    return xgs

                xg0 = emit_gathers(0, xlo_ext, "xg0")
                xg1 = emit_gathers(1, xhi_ext, "xg1")

                # second-half shared w2 (streams + expert w2 loads)
                shared_w2x(1)

                # pl zero-fill rides the now-idle sync DMA queue; must finish
                # before the first dma_scatter_add (mid expert phase)
                zb = cn.tile([P, D], dt.bfloat16)
                nc.vector.memset(zb[:], 0.0)
                for h in range(2):
                    pr = pl_d[h][:, :].rearrange("(a p) f -> p a f", p=P)
                    for g in range(HT // P):
                        nc.sync.dma_start(out=pr[:, g, :], in_=zb[:])

                def emit_scatters(h, obs):
                    for b in range(NB):
                        nc.gpsimd.dma_scatter_add(
                            pl_d[h][:, :], obs[b][:],
                            idxs16[h][:, 24 * b:24 * (b + 1)], B, B, D)

                def expert_compute(h, xgs):
                    obs = []
                    for b in range(NB):
                        xg = xgs[b]
                        for fk in range(16):
                            ph1 = ps.tile([P, 512], dt.float32, tag="mm",
                                          bufs=3, name="ph1")
                            for a in range(8):
                                nc.tensor.matmul(
                                    out=ph1[:, 0:B],
                                    lhsT=w1s[:, a, fk * P:(fk + 1) * P],
                                    rhs=xg[:, a, :], start=(a == 0),
                                    stop=(a == 7))
                            ph3 = ps.tile([P, 512], dt.float32, tag="mm",
                                          bufs=3, name="ph3")
                            for a in range(8):
                                nc.tensor.matmul(
                                    out=ph3[:, 0:B],
                                    lhsT=w3s[:, a, fk * P:(fk + 1) * P],
                                    rhs=xg[:, a, :], start=(a == 0),
                                    stop=(a == 7))
                            hg = ep.tile([P, B], dt.bfloat16, bufs=2,
                                         name="ehg")
                            nc.scalar.activation(out=hg[:], in_=ph1[:, 0:B],
                                                 func=AF.Silu)
                            h3b = ep.tile([P, B], dt.bfloat16, bufs=2,
                                          name="eh3b")
                            nc.scalar.activation(out=h3b[:], in_=ph3[:, 0:B],
                                                 func=AF.Copy)
                            nc.vector.tensor_mul(hbuf[:, fk, 0:B], hg[:],
                                                 h3b[:])
                        ob = ep.tile([P, NB, D], dt.bfloat16, bufs=2,
                                     name="ob")
                        for tc_ in range(NB):
                            for dh in range(2):
                                po = ps.tile([P, 512], dt.float32, tag="mm",
                                             bufs=3, name="po")
                                for fk in range(16):
                                    nc.tensor.matmul(
                                        out=po[:],
                                        lhsT=hbuf[:, fk, tc_ * P:(tc_ + 1) * P],
                                        rhs=w2s[:, fk,
                                                dh * 512:(dh + 1) * 512],
                                        start=(fk == 0), stop=(fk == 15))
                                nc.vector.tensor_scalar_mul(
                                    ob[:, tc_, dh * 512:(dh + 1) * 512], po[:],
                                    wcol[h][:, 3 * b + tc_:3 * b + tc_ + 1])
                        obs.append(ob)
                    return obs

                obs0 = expert_compute(0, xg0)
                emit_scatters(0, obs0)
                obs1 = expert_compute(1, xg1)
                # RS on half 0: emitted after half-1 PE work; on the gpsimd
                # queue it sits right after the h0 scatters so it triggers as
                # soon as pl0 is complete — overlapping half-1 compute.
                nc.gpsimd.collective_compute(
                    "ReduceScatter", OP.add, replica_groups=RG,
                    ins=[pl_d[0][:, :].opt()], outs=[rs_d[0][:, :].opt()])

                def emit_out(h):
                    # combine on gpsimd: its in-order queue position (after
                    # the RS) keeps the RS wait off the DVE/PE pipelines
                    for pair in range(2):
                        rsl = ep.tile([P, 2, D], dt.bfloat16, bufs=1,
                                      name="rsl")
                        nc.sync.dma_start(
                            out=rsl[:],
                            in_=rs_d[h][256 * pair:256 * (pair + 1), :]
                            .rearrange("(c p) f -> p c f", p=P))
                        for j in range(2):
                            tc_ = 2 * pair + j
                            of = ep.tile([P, D], dt.float32, bufs=1,
                                         name="of")
                            nc.gpsimd.tensor_tensor(
                                out=of[:], in0=rsl[:, j, :],
                                in1=out_s[:, 4 * h + tc_, :], op=OP.add)
                            nc.sync.dma_start(
                                out=out_ext[:, :]
                                .rearrange("(c p) f -> p c f", p=P)
                                [:, 4 * h + tc_, :],
                                in_=of[:])

                emit_out(0)
                emit_scatters(1, obs1)
                nc.gpsimd.collective_compute(
                    "ReduceScatter", OP.add, replica_groups=RG,
                    ins=[pl_d[1][:, :].opt()], outs=[rs_d[1][:, :].opt()])
                emit_out(1)

    nc.compile()
    _CACHE["nc"] = nc
    return nc


def _shard(inputs):
    bf16 = ml_dtypes.bfloat16
    x = np.ascontiguousarray(np.asarray(inputs["hidden_states"], np.float32))
    xbf = x.astype(bf16)
    # position p = 4096h + 512r + i  <->  token 1024r + 512h + i
    xperm = np.ascontiguousarray(
        xbf.reshape(8, 2, 512, D).transpose(1, 0, 2, 3).reshape(2, HT, D))
    gw9 = np.zeros((D, 16), np.float32)
    gw9[:, 0:8] = np.asarray(inputs["gate_w"], np.float32)
    gw9[:, 8:9] = np.asarray(inputs["sgate_w"], np.float32)
    w1 = np.asarray(inputs["w1"], np.float32).astype(bf16)
    w3 = np.asarray(inputs["w3"], np.float32).astype(bf16)
    w2 = np.asarray(inputs["w2"], np.float32).astype(bf16)
    sw1 = np.ascontiguousarray(np.asarray(inputs["sw1"], np.float32).astype(bf16))
    sw3 = np.ascontiguousarray(np.asarray(inputs["sw3"], np.float32).astype(bf16))
    sw2 = np.ascontiguousarray(np.asarray(inputs["sw2"], np.float32).astype(bf16))
    in_maps = []
    for r in range(8):
        own = slice(1024 * r, 1024 * (r + 1))
        in_maps.append(dict(
            xlo=xperm[0],
            xhi=xperm[1],
            xtr=np.ascontiguousarray(x[own].T),
            xtb=np.ascontiguousarray(xbf[own].T),
            gw9=gw9,
            w1e=np.ascontiguousarray(w1[r]),
            w3e=np.ascontiguousarray(w3[r]),
            w2e=np.ascontiguousarray(w2[r]),
            sw1e=sw1,
            sw3e=sw3,
            sw2e=sw2,
        ))
    return in_maps


def run(inputs, trace=False):
    nc = _build()
    in_maps = _shard(inputs)
    res = run_bass_kernel_spmd(nc, in_maps, list(range(8)), trace=trace)
    out = np.concatenate([res.results[r]["out"] for r in range(8)], axis=0)
    return out.astype(np.float32), res


def kernel(**inputs):
    out, _ = run(inputs, trace=False)
    return out


# revision 21
# speedup vs baseline: 1.5736x; 1.0314x over previous
"""MoE layer (moe_routing) Trainium2 Bass kernel — 8-core expert parallelism, v6.

Strategy (hardcoded for T=8192, D=1024, F=2048, E=8, top_k=2, 8 cores):
  - Core r owns expert r's w1/w3/w2 (host-precast bf16) and computes the full
    shared expert for its own 1024 tokens (token-sharded shared expert).
  - Router: core r routes its own tokens [1024r, 1024(r+1)) in float32r; the
    renormalized top-2 softmax weights are sigmoid(l1-l2) / 1-sigmoid(l1-l2).
    A small AllToAll ([8,1024] f32) sends each expert's combine-weight column
    to its owner core (out shard a = weights of core a's tokens, my expert).
  - Tokens are split into two halves by position (for split ReduceScatter):
    position p = 4096h + 512r + i  <->  token 1024r + 512h + i. Host permutes
    a bf16 copy of x into that layout (xlo/xhi gather sources).
  - Compaction per half: mask -> cumsum-by-triangular-matmul -> 32 indirect
    scatters of (local position, weight) pairs (the two halves' scatter
    chains are interleaved so their DMA round-trips overlap); then one
    transposed dma_gather per 384-block dispatches token rows and one
    dma_scatter_add per block combines weighted FFN rows into the half's
    bf16 partial.
  - FFN matmuls keep tokens in the free dim for w1/w3 and use the
    out[t,d] = h[f,t]^T @ w2[f,d] orientation for w2 (no transposes at all).
  - Two ReduceScatters (one per half): the first overlaps half-1 compute.
    Shared-expert output (SBUF-resident) is added after RS on gpsimd (so the
    RS wait cannot block the DVE pipeline); out is fp32.
  - Emission order tuned so the DMA queues serve the router/shared streams
    first; expert-weight loads are spread across the second-half stream
    slots; partial zero-fill rides an idle DMA window late.
"""
import sys

sys.path.insert(0, "/opt/trn_rl_repo")

import numpy as np
import ml_dtypes

import concourse.bacc as bacc
import concourse.mybir as mybir
import concourse.tile as tile
from concourse.bass import IndirectOffsetOnAxis
from concourse.bass_utils import run_bass_kernel_spmd
from concourse.masks import make_identity

dt = mybir.dt
AF = mybir.ActivationFunctionType
OP = mybir.AluOpType

P = 128
T, D, F, E = 8192, 1024, 2048, 8
TSL = 1024            # own tokens per core
HT = 4096             # tokens per half (global)
C2 = 1152             # expert token capacity per half (max measured 1118)
NCH = 32              # 128-chunks per half
B = 384               # expert FFN block (tokens per gather/scatter)
NB = 3                # blocks per half (3*384 = 1152)
BIG = 1 << 20
RG = [list(range(8))]

_CACHE = {}


def _build():
    if "nc" in _CACHE:
        return _CACHE["nc"]
    nc = bacc.Bacc("TRN2", target_bir_lowering=False, debug=False, num_devices=8)

    xlo_ext = nc.dram_tensor("xlo", [HT, D], dt.bfloat16, kind="ExternalInput")
    xhi_ext = nc.dram_tensor("xhi", [HT, D], dt.bfloat16, kind="ExternalInput")
    xtr_ext = nc.dram_tensor("xtr", [D, TSL], dt.float32, kind="ExternalInput")
    xtb_ext = nc.dram_tensor("xtb", [D, TSL], dt.bfloat16, kind="ExternalInput")
    gw9_ext = nc.dram_tensor("gw9", [D, 16], dt.float32, kind="ExternalInput")
    w1_ext = nc.dram_tensor("w1e", [D, F], dt.bfloat16, kind="ExternalInput")
    w3_ext = nc.dram_tensor("w3e", [D, F], dt.bfloat16, kind="ExternalInput")
    w2_ext = nc.dram_tensor("w2e", [F, D], dt.bfloat16, kind="ExternalInput")
    sw1_ext = nc.dram_tensor("sw1e", [D, F], dt.bfloat16, kind="ExternalInput")
    sw3_ext = nc.dram_tensor("sw3e", [D, F], dt.bfloat16, kind="ExternalInput")
    sw2_ext = nc.dram_tensor("sw2e", [F, D], dt.bfloat16, kind="ExternalInput")
    out_ext = nc.dram_tensor("out", [TSL, D], dt.float32, kind="ExternalOutput")

    with tile.TileContext(nc) as tc:
        with tc.tile_pool(name="cn", bufs=1) as cn, \
             tc.tile_pool(name="wk", bufs=2) as wk, \
             tc.tile_pool(name="ps", bufs=1, space="PSUM") as ps, \
             tc.tile_pool(name="dr", bufs=1, space="DRAM") as dr:

            # ---------------- DRAM scratch ----------------
            a2a_in = dr.tile([8, TSL], dt.float32)
            a2a_out = dr.tile([8, TSL], dt.float32)
            iw_d = [dr.tile([C2, 2], dt.int32, name=f"iw{i}") for i in range(2)]
            pl_d = [dr.tile([HT, D], dt.bfloat16, name=f"pl{i}") for i in range(2)]
            rs_d = [dr.tile([HT // 8, D], dt.bfloat16, name=f"rs{i}")
                    for i in range(2)]

            # ---------------- constants ----------------
            ident_f = cn.tile([NCH, NCH], dt.float32)
            make_identity(nc, ident_f[:])
            tri_bf = cn.tile([P, P], dt.bfloat16)
            nc.vector.memset(tri_bf[:], 1.0)
            nc.gpsimd.affine_select(
                out=tri_bf[:], in_=tri_bf[:], pattern=[[1, P]], base=-1,
                channel_multiplier=-1, compare_op=OP.is_ge, fill=0.0)
            ones_row_f = cn.tile([1, P], dt.float32)
            nc.vector.memset(ones_row_f[:], 1.0)
            iota8_i = cn.tile([P, E], dt.int32)
            nc.gpsimd.iota(iota8_i[:], pattern=[[1, E]], base=0,
                           channel_multiplier=0)
            iota8_f = cn.tile([P, E], dt.float32)
            nc.vector.tensor_copy(out=iota8_f[:], in_=iota8_i[:])
            # local position ids: chunk k=a*4+b holds ids 512a+128b+p
            iota_h = cn.tile([P, 8, 4], dt.int32)
            nc.gpsimd.iota(iota_h[:], pattern=[[512, 8], [128, 4]], base=0,
                           channel_multiplier=1)

            zi = cn.tile([P, C2 // P, 2], dt.int32)
            nc.vector.memset(zi[:], 0)

            gw9s = cn.tile([P, 8, 16], dt.float32r)
            nc.sync.dma_start(
                out=gw9s[:],
                in_=gw9_ext[:, :].bitcast(dt.float32r)
                .rearrange("(a p) f -> p a f", p=P))

            w1s = cn.tile([P, 8, F], dt.bfloat16)
            w3s = cn.tile([P, 8, F], dt.bfloat16)
            w2s = cn.tile([P, 16, D], dt.bfloat16)
            hbuf = cn.tile([P, 16, 512], dt.bfloat16)
            out_s = cn.tile([P, 8, D], dt.bfloat16)
            payload = cn.tile([P, 8, 9], dt.float32)
            idxs16 = []
            wcol = []

            # ======== phase pool: router + shared expert ========
            with tc.tile_pool(name="sp", bufs=1) as sp:
                xtb = sp.tile([P, 8, TSL], dt.bfloat16, name="xtb")

                # ---- router on own tokens ----
                for c in range(8):
                    xrt = sp.tile([P, 8, P], dt.float32r, bufs=2, name="xrt")
                    nc.sync.dma_start(
                        out=xrt[:],
                        in_=xtr_ext[:, c * P:(c + 1) * P].bitcast(dt.float32r)
                        .rearrange("(a p) f -> p a f", p=P))
                    lg9 = ps.tile([P, 16], dt.float32, tag="sm", bufs=1,
                                  name="lg9")
                    for a in range(8):
                        nc.tensor.matmul(out=lg9[:], lhsT=xrt[:, a, :],
                                         rhs=gw9s[:, a, :],
                                         start=(a == 0), stop=(a == 7))
                    lgc = sp.tile([P, 9], dt.float32, bufs=2, name="lgc")
                    nc.vector.tensor_copy(out=lgc[:], in_=lg9[:, 0:9])
                    mx = sp.tile([P, 8], dt.float32, bufs=2, name="mx")
                    nc.vector.max(out=mx[:], in_=lgc[:, 0:8])
                    mi = sp.tile([P, 8], dt.uint32, bufs=2, name="mi")
                    nc.vector.max_index(out=mi[:], in_max=mx[:],
                                        in_values=lgc[:, 0:8])
                    mif = sp.tile([P, 2], dt.float32, bufs=2, name="mif")
                    nc.vector.tensor_copy(out=mif[:],
                                          in_=mi[:, 0:2].bitcast(dt.int32))
                    d12 = sp.tile([P, 1], dt.float32, bufs=2, name="d12")
                    nc.vector.tensor_sub(d12[:], mx[:, 0:1], mx[:, 1:2])
                    wA = sp.tile([P, 1], dt.float32, bufs=2, name="wA")
                    nc.scalar.activation(out=wA[:], in_=d12[:], func=AF.Sigmoid)
                    wB = sp.tile([P, 1], dt.float32, bufs=2, name="wB")
                    nc.scalar.activation(out=wB[:], in_=wA[:], func=AF.Copy,
                                         scale=-1.0, bias=1.0)
                    eq1 = sp.tile([P, 8], dt.float32, bufs=2, name="eq1")
                    nc.vector.tensor_tensor(
                        out=eq1[:], in0=mif[:, 0:1].to_broadcast([P, 8]),
                        in1=iota8_f[:], op=OP.is_equal)
                    eq2 = sp.tile([P, 8], dt.float32, bufs=2, name="eq2")
                    nc.vector.tensor_tensor(
                        out=eq2[:], in0=mif[:, 1:2].to_broadcast([P, 8]),
                        in1=iota8_f[:], op=OP.is_equal)
                    nc.vector.tensor_tensor(out=eq1[:], in0=eq1[:],
                                            in1=wA[:].to_broadcast([P, 8]),
                                            op=OP.mult)
                    nc.vector.tensor_tensor(out=eq2[:], in0=eq2[:],
                                            in1=wB[:].to_broadcast([P, 8]),
                                            op=OP.mult)
                    nc.vector.tensor_add(payload[:, c, 0:8], eq1[:], eq2[:])
                    nc.scalar.activation(out=payload[:, c, 8:9],
                                         in_=lgc[:, 8:9], func=AF.Sigmoid)

                for a in range(8):
                    nc.sync.dma_start(out=xtb[:, a, :],
                                      in_=xtb_ext[a * P:(a + 1) * P, :])
                # iw zero-init (tiny; pl zero-fill is emitted late)
                for h in range(2):
                    nc.sync.dma_start(
                        out=iw_d[h][:, :].rearrange("(a p) f -> p a f", p=P),
                        in_=zi[:])

                def shared_w13(h):
                    # h_s[f, t] for own tokens [512h, 512h+512); expert w1/w3
                    # resident loads are spread across the h1 stream slots
                    t0 = 512 * h
                    for fk in range(16):
                        s1t = sp.tile([P, 8, P], dt.bfloat16, bufs=3,
                                      name="s1t")
                        nc.sync.dma_start(
                            out=s1t[:],
                            in_=sw1_ext[:, fk * P:(fk + 1) * P]
                            .rearrange("(a p) f -> p a f", p=P))
                        s3t = sp.tile([P, 8, P], dt.bfloat16, bufs=3,
                                      name="s3t")
                        nc.sync.dma_start(
                            out=s3t[:],
                            in_=sw3_ext[:, fk * P:(fk + 1) * P]
                            .rearrange("(a p) f -> p a f", p=P))
                        if h == 1:
                            if fk < 8:
                                nc.sync.dma_start(
                                    out=w1s[:, fk, :],
                                    in_=w1_ext[fk * P:(fk + 1) * P, :])
                            else:
                                a2 = fk - 8
                                nc.sync.dma_start(
                                    out=w3s[:, a2, :],
                                    in_=w3_ext[a2 * P:(a2 + 1) * P, :])
                        ph1 = ps.tile([P, 512], dt.float32, tag="mm", bufs=3,
                                      name="ph1")
                        for a in range(8):
                            nc.tensor.matmul(out=ph1[:], lhsT=s1t[:, a, :],
                                             rhs=xtb[:, a, t0:t0 + 512],
                                             start=(a == 0), stop=(a == 7))
                        ph3 = ps.tile([P, 512], dt.float32, tag="mm", bufs=3,
                                      name="ph3")
                        for a in range(8):
                            nc.tensor.matmul(out=ph3[:], lhsT=s3t[:, a, :],
                                             rhs=xtb[:, a, t0:t0 + 512],
                                             start=(a == 0), stop=(a == 7))
                        hg = sp.tile([P, 512], dt.bfloat16, bufs=2, name="hg")
                        nc.scalar.activation(out=hg[:], in_=ph1[:],
                                             func=AF.Silu)
                        h3b = sp.tile([P, 512], dt.bfloat16, bufs=2,
                                      name="h3b")
                        nc.scalar.activation(out=h3b[:], in_=ph3[:],
                                             func=AF.Copy)
                        nc.vector.tensor_mul(hbuf[:, fk, 0:512], hg[:],
                                             h3b[:])

                def shared_w2x(h):
                    # stream sw2; expert w2 resident loads spread across the
                    # h1 slots; 4 token-chunk accumulators live per dh
                    for dh in range(2):
                        pos = [ps.tile([P, 512], dt.float32, tag="po4",
                                       bufs=4, name="pos")
                               for _ in range(4)]
                        for fk in range(16):
                            s2t = wk.tile([P, 512], dt.bfloat16, bufs=3,
                                          name="s2t")
                            nc.sync.dma_start(
                                out=s2t[:],
                                in_=sw2_ext[fk * P:(fk + 1) * P,
                                            dh * 512:(dh + 1) * 512])
                            if h == 1 and fk % 2 == 0:
                                a2 = dh * 8 + fk // 2
                                nc.sync.dma_start(
                                    out=w2s[:, a2, :],
                                    in_=w2_ext[a2 * P:(a2 + 1) * P, :])
                            for tc_ in range(4):
                                nc.tensor.matmul(
                                    out=pos[tc_][:],
                                    lhsT=hbuf[:, fk, tc_ * P:(tc_ + 1) * P],
                                    rhs=s2t[:],
                                    start=(fk == 0), stop=(fk == 15))
                        for tc_ in range(4):
                            nc.vector.tensor_scalar_mul(
                                out_s[:, 4 * h + tc_, dh * 512:(dh + 1) * 512],
                                pos[tc_][:], payload[:, 4 * h + tc_, 8:9])

                shared_w13(0)
                shared_w2x(0)

                # AllToAll (extraction + collective + cwe loads on gpsimd)
                for e in range(8):
                    nc.gpsimd.dma_start(
                        out=a2a_in[e:e + 1, :].rearrange("o (c p) -> p (o c)",
                                                         p=P),
                        in_=payload[:, :, e])
                nc.gpsimd.collective_compute(
                    "AllToAll", OP.bypass, replica_groups=RG,
                    ins=[a2a_in[:, :].opt()], outs=[a2a_out[:, :].opt()])
                cwes = []
                for h in range(2):
                    cwe = cn.tile([P, NCH], dt.float32, name=f"cwe{h}")
                    for a in range(8):
                        nc.gpsimd.dma_start(
                            out=cwe[:, 4 * a:4 * (a + 1)],
                            in_=a2a_out[a:a + 1, 512 * h:512 * h + 512]
                            .rearrange("o (c p) -> p (o c)", p=P))
                    cwes.append(cwe)

                # anchor: comp matmuls use ones_late (produced from the first
                # shared-w2 evac) so the scheduler cannot place them in the
                # tensor stream before the shared w2 stage has begun (the
                # real A2A latency would stall the PE there).
                ones_late = cn.tile([P, 1], dt.bfloat16)
                nc.vector.tensor_scalar(out=ones_late[:],
                                        in0=out_s[:, 0, 0:1], scalar1=0.0,
                                        scalar2=1.0, op0=OP.mult, op1=OP.add)

                def compact_prep(h):
                    cwe = cwes[h]
                    mask_f = cn.tile([P, NCH], dt.float32, name=f"maskf{h}")
                    nc.vector.tensor_scalar(out=mask_f[:], in0=cwe[:],
                                            scalar1=0.0, scalar2=None,
                                            op0=OP.is_gt)
                    mask_bf = cn.tile([P, NCH], dt.bfloat16, name=f"maskb{h}")
                    nc.vector.tensor_copy(out=mask_bf[:], in_=mask_f[:])

                    pcst = ps.tile([P, 1], dt.float32, tag="sm", bufs=1,
                                   name="pcst")
                    nc.tensor.matmul(out=pcst[0:NCH, :], lhsT=mask_bf[:],
                                     rhs=ones_late[:], start=True, stop=True)
                    cst = wk.tile([NCH, 1], dt.bfloat16, bufs=2, name="cst")
                    nc.vector.tensor_copy(out=cst[:], in_=pcst[0:NCH, :])
                    ppre = ps.tile([P, 1], dt.float32, tag="sm", bufs=1,
                                   name="ppre")
                    nc.tensor.matmul(out=ppre[0:NCH, :],
                                     lhsT=tri_bf[0:NCH, 0:NCH], rhs=cst[:],
                                     start=True, stop=True)
                    pre_sb = wk.tile([NCH, 1], dt.float32, bufs=2,
                                     name="pre_sb")
                    nc.vector.tensor_copy(out=pre_sb[:], in_=ppre[0:NCH, :])
                    pprer = ps.tile([1, NCH], dt.float32, tag="sm", bufs=1,
                                    name="pprer")
                    nc.tensor.transpose(out=pprer[:], in_=pre_sb[:],
                                        identity=ident_f[:])
                    pre_row = wk.tile([1, NCH], dt.float32, bufs=2,
                                      name="pre_row")
                    nc.vector.tensor_copy(out=pre_row[:], in_=pprer[:])

                    ppos = ps.tile([P, NCH], dt.float32, tag="sm", bufs=1,
                                   name="ppos")
                    nc.tensor.matmul(out=ppos[:], lhsT=tri_bf[:],
                                     rhs=mask_bf[:], start=True, stop=False)
                    nc.tensor.matmul(out=ppos[:], lhsT=ones_row_f[:],
                                     rhs=pre_row[:], start=False, stop=True)
                    posm = wk.tile([P, NCH], dt.float32, bufs=2, name="posm")
                    nc.vector.tensor_tensor(out=posm[:], in0=ppos[:],
                                            in1=mask_f[:], op=OP.mult)
                    dump = wk.tile([P, NCH], dt.float32, bufs=2, name="dump")
                    nc.vector.tensor_scalar(out=dump[:], in0=mask_f[:],
                                            scalar1=float(-BIG),
                                            scalar2=float(BIG),
                                            op0=OP.mult, op1=OP.add)
                    nc.vector.tensor_add(posm[:], posm[:], dump[:])
                    o_i = cn.tile([P, NCH], dt.int32, name=f"oi{h}")
                    nc.vector.tensor_copy(out=o_i[:], in_=posm[:])

                    iw_pack = cn.tile([P, NCH, 2], dt.int32, name=f"iwp{h}")
                    for a in range(8):
                        nc.vector.tensor_copy(
                            out=iw_pack[:, 4 * a:4 * (a + 1), 0],
                            in_=iota_h[:, a, :])
                    nc.vector.tensor_copy(out=iw_pack[:, :, 1],
                                          in_=cwe[:].bitcast(dt.int32))
                    return o_i, iw_pack

                oi0, iwp0 = compact_prep(0)
                oi1, iwp1 = compact_prep(1)
                # interleave the halves' scatters: two independent WAW chains
                for k in range(NCH):
                    nc.gpsimd.indirect_dma_start(
                        out=iw_d[0][:, :],
                        out_offset=IndirectOffsetOnAxis(ap=oi0[:, k:k + 1],
                                                        axis=0),
                        in_=iwp0[:, k, :], in_offset=None,
                        bounds_check=C2 - 1, oob_is_err=False)
                    nc.gpsimd.indirect_dma_start(
                        out=iw_d[1][:, :],
                        out_offset=IndirectOffsetOnAxis(ap=oi1[:, k:k + 1],
                                                        axis=0),
                        in_=iwp1[:, k, :], in_offset=None,
                        bounds_check=C2 - 1, oob_is_err=False)

                for h in range(2):
                    idx = cn.tile([P, C2 // 16], dt.int16, name=f"idx{h}")
                    for grp in range(8):
                        nc.gpsimd.dma_start(
                            out=idx[grp * 16:(grp + 1) * 16, :],
                            in_=iw_d[h][:, :].bitcast(dt.int16)[:, 0:1]
                            .rearrange("(s p) f -> p (s f)", p=16))
                    idxs16.append(idx)
                    wc = cn.tile([P, C2 // P], dt.float32, name=f"wc{h}")
                    nc.gpsimd.dma_start(
                        out=wc[:],
                        in_=iw_d[h][:, :].bitcast(dt.float32)[:, 1:2]
                        .rearrange("(c p) f -> p (c f)", p=P))
                    wcol.append(wc)


                shared_w13(1)

            # ======== phase pool: compaction + expert FFN + output ========
            with tc.tile_pool(name="ep", bufs=1) as ep:
                def emit_gathers(h, xsrc, name):
                    xgs = []
                    for b in range(NB):
                        xg = ep.tile([P, 8, B], dt.bfloat16, bufs=2, name=name)
                        nc.gpsimd.dma_gather(
                            xg[:], xsrc[:, :],
                            idxs16[h][:, 24 * b:24 * (b + 1)],
                            B, B, D, transpose=True)
                        xgs.append(xg)
                    return xgs

                xg0 = emit_gathers(0, xlo_ext, "xg0")
                xg1 = emit_gathers(1, xhi_ext, "xg1")

                # second-half shared w2 (streams + expert w2 loads)
                shared_w2x(1)

                # pl zero-fill rides the now-idle sync DMA queue; must finish
                # before the first dma_scatter_add (mid expert phase)
                zb = cn.tile([P, D], dt.bfloat16)
                nc.vector.memset(zb[:], 0.0)
                for h in range(2):
                    pr = pl_d[h][:, :].rearrange("(a p) f -> p a f", p=P)
                    for g in range(HT // P):
                        nc.sync.dma_start(out=pr[:, g, :], in_=zb[:])

                def emit_scatters(h, obs):
                    for b in range(NB):
                        nc.gpsimd.dma_scatter_add(
                            pl_d[h][:, :], obs[b][:],
                            idxs16[h][:, 24 * b:24 * (b + 1)], B, B, D)

                def expert_compute(h, xgs):
                    obs = []
                    for b in range(NB):
                        xg = xgs[b]
                        for fk in range(16):
                            ph1 = ps.tile([P, 512], dt.float32, tag="mm",
                                          bufs=3, name="ph1")
                            for a in range(8):
                                nc.tensor.matmul(
                                    out=ph1[:, 0:B],
                                    lhsT=w1s[:, a, fk * P:(fk + 1) * P],
                                    rhs=xg[:, a, :], start=(a == 0),
                                    stop=(a == 7))
                            ph3 = ps.tile([P, 512], dt.float32, tag="mm",
                                          bufs=3, name="ph3")
                            for a in range(8):
                                nc.tensor.matmul(
                                    out=ph3[:, 0:B],
                                    lhsT=w3s[:, a, fk * P:(fk + 1) * P],
                                    rhs=xg[:, a, :], start=(a == 0),
                                    stop=(a == 7))
                            hg = ep.tile([P, B], dt.bfloat16, bufs=2,
                                         name="ehg")
                            nc.scalar.activation(out=hg[:], in_=ph1[:, 0:B],
                                                 func=AF.Silu)
                            h3b = ep.tile([P, B], dt.bfloat16, bufs=2,
                                          name="eh3b")
                            nc.scalar.activation(out=h3b[:], in_=ph3[:, 0:B],
                                                 func=AF.Copy)
                            nc.vector.tensor_mul(hbuf[:, fk, 0:B], hg[:],
                                                 h3b[:])
                        ob = ep.tile([P, NB, D], dt.bfloat16, bufs=2,
                                     name="ob")
                        for tc_ in range(NB):
                            for dh in range(2):
                                po = ps.tile([P, 512], dt.float32, tag="mm",
                                             bufs=3, name="po")
                                for fk in range(16):
                                    nc.tensor.matmul(
                                        out=po[:],
                                        lhsT=hbuf[:, fk, tc_ * P:(tc_ + 1) * P],
                                        rhs=w2s[:, fk,
                                                dh * 512:(dh + 1) * 512],
                                        start=(fk == 0), stop=(fk == 15))
                                nc.vector.tensor_scalar_mul(
                                    ob[:, tc_, dh * 512:(dh + 1) * 512], po[:],
                                    wcol[h][:, 3 * b + tc_:3 * b + tc_ + 1])
                        obs.append(ob)
                    return obs

                obs0 = expert_compute(0, xg0)
                emit_scatters(0, obs0)
                obs1 = expert_compute(1, xg1)
                # RS on half 0: emitted after half-1 PE work; on the gpsimd
                # queue it sits right after the h0 scatters so it triggers as
                # soon as pl0 is complete — overlapping half-1 compute.
                nc.gpsimd.collective_compute(
                    "ReduceScatter", OP.add, replica_groups=RG,
                    ins=[pl_d[0][:, :].opt()], outs=[rs_d[0][:, :].opt()])

                def emit_out(h):
                    # combine on gpsimd: its in-order queue position (after
                    # the RS) keeps the RS wait off the DVE/PE pipelines
                    for pair in range(2):
                        rsl = ep.tile([P, 2, D], dt.bfloat16, bufs=1,
                                      name="rsl")
                        nc.sync.dma_start(
                            out=rsl[:],
                            in_=rs_d[h][256 * pair:256 * (pair + 1), :]
                            .rearrange("(c p) f -> p c f", p=P))
                        for j in range(2):
                            tc_ = 2 * pair + j
                            of = ep.tile([P, D], dt.float32, bufs=1,
                                         name="of")
                            nc.gpsimd.tensor_tensor(
                                out=of[:], in0=rsl[:, j, :],
                                in1=out_s[:, 4 * h + tc_, :], op=OP.add)
                            nc.sync.dma_start(
                                out=out_ext[:, :]
                                .rearrange("(c p) f -> p c f", p=P)
                                [:, 4 * h + tc_, :],
                                in_=of[:])

                emit_out(0)
                emit_scatters(1, obs1)
                nc.gpsimd.collective_compute(
                    "ReduceScatter", OP.add, replica_groups=RG,
                    ins=[pl_d[1][:, :].opt()], outs=[rs_d[1][:, :].opt()])
                emit_out(1)

    nc.compile()
    _CACHE["nc"] = nc
    return nc


def _shard(inputs):
    bf16 = ml_dtypes.bfloat16
    x = np.ascontiguousarray(np.asarray(inputs["hidden_states"], np.float32))
    xbf = x.astype(bf16)
    # position p = 4096h + 512r + i  <->  token 1024r + 512h + i
    xperm = np.ascontiguousarray(
        xbf.reshape(8, 2, 512, D).transpose(1, 0, 2, 3).reshape(2, HT, D))
    gw9 = np.zeros((D, 16), np.float32)
    gw9[:, 0:8] = np.asarray(inputs["gate_w"], np.float32)
    gw9[:, 8:9] = np.asarray(inputs["sgate_w"], np.float32)
    w1 = np.asarray(inputs["w1"], np.float32).astype(bf16)
    w3 = np.asarray(inputs["w3"], np.float32).astype(bf16)
    w2 = np.asarray(inputs["w2"], np.float32).astype(bf16)
    sw1 = np.ascontiguousarray(np.asarray(inputs["sw1"], np.float32).astype(bf16))
    sw3 = np.ascontiguousarray(np.asarray(inputs["sw3"], np.float32).astype(bf16))
    sw2 = np.ascontiguousarray(np.asarray(inputs["sw2"], np.float32).astype(bf16))
    in_maps = []
    for r in range(8):
        own = slice(1024 * r, 1024 * (r + 1))
        in_maps.append(dict(
            xlo=xperm[0],
            xhi=xperm[1],
            xtr=np.ascontiguousarray(x[own].T),
            xtb=np.ascontiguousarray(xbf[own].T),
            gw9=gw9,
            w1e=np.ascontiguousarray(w1[r]),
            w3e=np.ascontiguousarray(w3[r]),
            w2e=np.ascontiguousarray(w2[r]),
            sw1e=sw1,
            sw3e=sw3,
            sw2e=sw2,
        ))
    return in_maps


def run(inputs, trace=False):
    nc = _build()
    in_maps = _shard(inputs)
    res = run_bass_kernel_spmd(nc, in_maps, list(range(8)), trace=trace)
    out = np.concatenate([res.results[r]["out"] for r in range(8)], axis=0)
    return out.astype(np.float32), res


def kernel(**inputs):
    out, _ = run(inputs, trace=False)
    return out
